# revision 11
# baseline (speedup 1.0000x reference)
"""Trainium2 Bass kernel for nn_Attention (llama-style attention block, GQA, RoPE).

v2 — bf16 dataflow (rel-err gate 2e-2; bf16 lands ~1e-3):
  - All matmul operands bf16 (PSUM accumulation stays f32): same PE cycle
    count as f32r but half the HBM/DMA traffic everywhere.
  - Projection runs 1024-token blocks with 1024-wide moving operands
    (PSUM tiles spanning 2 banks), halving PE instruction count.
  - Softmax denominators no longer burn PE matmul cycles per score chunk:
    a bf16 DVE accumulator (4x mode) sums the exp tiles, one tiny
    ones-matmul per q-block reduces it across partitions.
  - Normalization moved to the sender side of the AllToAll (reciprocal on
    DVE, partition_broadcast on Pool) so the wo stage consumes a2a output
    directly - no post-collective normalize pass on the critical path.
  - Causal mask applied multiplicatively to the bf16 exp tiles (DVE 4x)
    instead of f32 adds on PSUM.
  - V transposes via the DMA XBAR (16-bit transpose) instead of PE.
  - The batch-0 AllToAll overlaps batch-1 attention; sender-side
    normalization leaves only the batch-1 collective exposed.

Distribution (8 NeuronCores, Megatron-style tensor parallel over heads):
  - Each core gets 4 Q heads + its matching 1 KV head (wq/wk/wv output-dim
    sharded). Attention computed per-core in a transposed dataflow
    (head_dim on partitions, tokens on the free dim).
  - Per-batch AllToAll reshards the (already normalized) attention output
    token-parallel; each core then runs wo for its 2x256-token block
    against the full wo, so no AllReduce is needed.
"""

import sys

if "/opt/trn_rl_repo" not in sys.path:
    sys.path.insert(0, "/opt/trn_rl_repo")

import numpy as np
import ml_dtypes

BF16 = ml_dtypes.bfloat16

N_CORES = 8
B, S, D = 2, 2048, 4096
N_HEADS = 32
N_KV_HEADS = 8
HEAD_DIM = 128
H_PER_CORE = N_HEADS // N_CORES          # 4 q heads per core
TOK = B * S                              # 4096 flattened tokens
QKV_M = H_PER_CORE * HEAD_DIM + 2 * HEAD_DIM  # 768 projection rows per core
PROJ_TOK = 512                           # token block in the projection stage
SQ_BLK = 512                             # moving width in attention (2 heads x 256)
A2A_TOK = 256                            # tokens per rank per per-batch AllToAll
NJJ = S // A2A_TOK                       # 8 q-blocks of 256 per batch
N_TCHUNK = S // HEAD_DIM                 # 16 key chunks per batch
SCALE = 1.0 / float(np.sqrt(HEAD_DIM))
NKC = D // 128                           # 32 contraction chunks

# partition permutation for RoPE: pair (even, odd) lives 16 partitions apart
# inside a 32-partition quadrant, so the rotation is a single stream_shuffle.
_P = np.arange(128)
_I_OF_P = 16 * (_P // 32) + (_P % 32) % 16          # rope pair index 0..63
_IS_ODD = (_P % 32) >= 16
PERM = (2 * _I_OF_P + _IS_ODD.astype(np.int64)).astype(np.int64)  # orig row in head block
SHUF_MASK = [(i + 16) % 32 for i in range(32)]

_PROGRAMS = {}


def _build_program(mask_mode):
    """Build + compile the SPMD program. mask_mode in {'causal', 'none', 'general'}."""
    import concourse.bass as bass
    import concourse.mybir as mybir
    import concourse.tile as tile
    from concourse import bacc

    f32 = mybir.dt.float32
    bf16 = mybir.dt.bfloat16
    Exp = mybir.ActivationFunctionType.Exp

    nc = bacc.Bacc("TRN2", target_bir_lowering=False, debug=False,
                   num_devices=N_CORES)

    xT = nc.dram_tensor("xT", [D, TOK], bf16, kind="ExternalInput")
    wqkvT = nc.dram_tensor("wqkvT", [D, QKV_M], bf16, kind="ExternalInput")
    # wo pre-tiled on host: [m_chunk, p, k_chunk, m_col] so each stationary
    # column-block DMA reads contiguous lines
    woT4 = nc.dram_tensor("woT4", [NKC, 128, NKC, 128], bf16, kind="ExternalInput")
    cos2 = nc.dram_tensor("cos2", [128, S], bf16, kind="ExternalInput")
    sin2 = nc.dram_tensor("sin2", [128, S], bf16, kind="ExternalInput")
    if mask_mode == "general":
        # additive mask stored transposed: maskT[k_pos, q_pos]
        maskT = nc.dram_tensor("maskT", [S, S], f32, kind="ExternalInput")
    out_d = nc.dram_tensor("out", [D, SQ_BLK], bf16, kind="ExternalOutput")

    xT_t = xT.ap().rearrange("(k p) t -> p k t", p=128)      # [128, 32, TOK]
    wqkvT_t = wqkvT.ap().rearrange("(k p) m -> p k m", p=128)  # [128, 32, 768]

    with tile.TileContext(nc) as tc:
        # at_kt/at_kv/at_q are hoisted alongside the projection pools so the
        # attention preamble DMAs (kT, v transposes, q loads) prefetch during
        # the projection stage instead of serializing at the pool boundary.
        with tc.tile_pool(name="const", bufs=1) as const, \
             tc.tile_pool(name="dram", bufs=1, space="DRAM") as dram, \
             tc.tile_pool(name="at_kt", bufs=2) as at_kt, \
             tc.tile_pool(name="at_kv", bufs=2) as at_kv, \
             tc.tile_pool(name="at_q", bufs=2) as at_q:
            # per-core q/k/v (transposed layout), split per batch
            qd = [dram.tile([H_PER_CORE * 128, S], bf16, name=f"qd{b_}")
                  for b_ in range(B)]
            kd = [dram.tile([128, S], bf16, name=f"kd{b_}") for b_ in range(B)]
            vd = [dram.tile([128, S], bf16, name=f"vd{b_}") for b_ in range(B)]
            # a2a payloads split per head-pair so each collective can fire as
            # soon as its two heads finish (hp half 0 -> heads 0,1)
            a2a_in = [[dram.tile([N_CORES, 256, A2A_TOK], bf16,
                                 name=f"a2a_in{b_}_{hp}") for hp in range(2)]
                      for b_ in range(B)]
            a2a_out = [[dram.tile([N_CORES, 256, A2A_TOK], bf16,
                                  name=f"a2a_out{b_}_{hp}") for hp in range(2)]
                       for b_ in range(B)]

            ones_col = const.tile([128, 1], bf16)     # lhsT for column sums
            nc.vector.memset(ones_col[:], 1.0)
            if mask_mode == "causal":
                # multiplicative 0/1 mask for the diagonal chunk-group:
                # cm[p, a, hh, t] = (t - p >= 128*a), same for both packed
                # heads hh
                cm = const.tile([128, 2, 2, A2A_TOK], bf16, name="cm")
                nc.gpsimd.memset(cm[:], 1.0)
                for a in range(2):
                    for hh in range(2):
                        nc.gpsimd.affine_select(
                            out=cm[:, a, hh, :],
                            in_=cm[:, a, hh, :],
                            pattern=[[1, A2A_TOK]], base=-128 * a,
                            channel_multiplier=-1,
                            compare_op=mybir.AluOpType.is_ge, fill=0.0,
                        )

            # ---------------- stage 1: fused QKV projection + RoPE ----------------
            # k-outer / m-inner with 6 live PSUM accumulation groups, so the
            # two half-K x tiles (xA, xB) double-buffer against each other.
            n_blk = TOK // PROJ_TOK   # 8
            HK = NKC // 2
            with tc.tile_pool(name="pj_w", bufs=1) as pj_w, \
                 tc.tile_pool(name="pj_x", bufs=2) as pj_x, \
                 tc.tile_pool(name="pj_cs", bufs=2) as pj_cs, \
                 tc.tile_pool(name="pj_t", bufs=2) as pj_t, \
                 tc.tile_pool(name="pj_o", bufs=2) as pj_o, \
                 tc.tile_pool(name="pj_ps", bufs=8, space="PSUM") as pj_ps:
                w_sb = pj_w.tile([128, NKC, QKV_M], bf16)
                x0 = slice(0, PROJ_TOK)
                xA0 = pj_x.tile([128, HK, PROJ_TOK], bf16, tag="xA")
                xB0 = pj_x.tile([128, HK, PROJ_TOK], bf16, tag="xB")
                nc.sync.dma_start(xA0[:], xT_t[:, 0:HK, x0])
                # split the weight load by k-chunk so the first matmuls can
                # start before the full 6.3MB arrives
                for kw in range(0, NKC, 4):
                    nc.sync.dma_start(w_sb[:, kw:kw + 4, :],
                                      wqkvT_t[:, kw:kw + 4, :])
                    if kw == 0:
                        nc.sync.dma_start(xB0[:], xT_t[:, HK:NKC, x0])
                for n in range(n_blk):
                    s0 = (n * PROJ_TOK) % S  # position within the batch
                    bn = n // (S // PROJ_TOK)  # batch of this token block
                    cols = slice(n * PROJ_TOK, (n + 1) * PROJ_TOK)
                    bcols = slice(s0, s0 + PROJ_TOK)
                    if n == 0:
                        xA, xB = xA0, xB0
                    else:
                        xA = pj_x.tile([128, HK, PROJ_TOK], bf16, tag="xA")
                        xB = pj_x.tile([128, HK, PROJ_TOK], bf16, tag="xB")
                        nc.sync.dma_start(xA[:], xT_t[:, 0:HK, cols])
                        nc.sync.dma_start(xB[:], xT_t[:, HK:NKC, cols])
                    c_sb = pj_cs.tile([128, PROJ_TOK], bf16, tag="c")
                    s_sb = pj_cs.tile([128, PROJ_TOK], bf16, tag="s")
                    nc.sync.dma_start(c_sb[:], cos2.ap()[:, s0:s0 + PROJ_TOK])
                    nc.sync.dma_start(s_sb[:], sin2.ap()[:, s0:s0 + PROJ_TOK])
                    pss = [pj_ps.tile([128, PROJ_TOK], f32, tag="ps",
                                      name=f"ps_{n}_{mi}")
                           for mi in range(QKV_M // 128)]
                    for k in range(NKC):
                        xsb = xA if k < HK else xB
                        xi = k if k < HK else k - HK
                        for m in range(QKV_M // 128):
                            nc.tensor.matmul(
                                pss[m][:], w_sb[:, k, m * 128:(m + 1) * 128],
                                xsb[:, xi, :],
                                start=(k == 0), stop=(k == NKC - 1))
                    for m in range(QKV_M // 128):  # q0..q3, k, v
                        ps = pss[m]
                        o_sb = pj_o.tile([128, PROJ_TOK], bf16, tag="o")
                        if m < 5:  # rope for q heads + k
                            tmp = pj_t.tile([128, PROJ_TOK], bf16, tag="tmp")
                            rot = pj_t.tile([128, PROJ_TOK], bf16, tag="rot")
                            t1 = pj_t.tile([128, PROJ_TOK], bf16, tag="t1")
                            nc.scalar.copy(tmp[:], ps[:])
                            nc.vector.stream_shuffle(rot[:], tmp[:], SHUF_MASK)
                            nc.vector.tensor_mul(t1[:], tmp[:], c_sb[:])
                            nc.vector.tensor_mul(rot[:], rot[:], s_sb[:])
                            nc.vector.tensor_add(o_sb[:], t1[:], rot[:])
                        else:
                            nc.scalar.copy(o_sb[:], ps[:])
                        if m < 4:
                            dst = qd[bn][m * 128:(m + 1) * 128, bcols]
                        elif m == 4:
                            dst = kd[bn][:, bcols]
                        else:
                            dst = vd[bn][:, bcols]
                        nc.sync.dma_start(dst, o_sb[:])

            # ---------------- stage 2: attention + AllToAlls ----------------
            # Two q-heads packed side by side in the 512-wide moving operand
            # (2 x 256 tokens); causality handled at 256-token granularity.
            # wo_w is opened before the attention pools so the wo weight
            # prefetch streams during attention / the collectives.
            with tc.tile_pool(name="wo_w", bufs=4) as wo_w:
                with tc.tile_pool(name="at_e", bufs=6) as at_e, \
                     tc.tile_pool(name="at_acc", bufs=2) as at_acc, \
                     tc.tile_pool(name="at_nrm", bufs=2) as at_nrm, \
                     tc.tile_pool(name="at_o", bufs=3) as at_o, \
                     tc.tile_pool(name="at_mt", bufs=4) as at_mt, \
                     tc.tile_pool(name="ps_s", bufs=2, space="PSUM") as ps_s, \
                     tc.tile_pool(name="ps_av", bufs=2, space="PSUM") as ps_av, \
                     tc.tile_pool(name="ps_sm", bufs=2, space="PSUM") as ps_sm:
                    for b in range(B):
                        kT = at_kt.tile([128, S], bf16, tag="kT")
                        nc.sync.dma_start(kT[:], kd[b][:])
                        v_nat = at_kv.tile([128, N_TCHUNK, 128], bf16, tag="vn")
                        for i in range(N_TCHUNK):
                            nc.sync.dma_start(v_nat[:, i, :],
                                              vd[b][:, i * 128:(i + 1) * 128],
                                              transpose=True)
                        for hp in range(2):
                            # q for heads (2hp, 2hp+1), interleaved as
                            # [p, jj, hh, t] so each jj reads a contiguous
                            # [128, 512] block covering both heads
                            qTp = at_q.tile([128, NJJ, 2, A2A_TOK], bf16,
                                            tag="qTp")
                            for hh in range(2):
                                nc.sync.dma_start(
                                    qTp[:, :, hh, :],
                                    qd[b][(2 * hp + hh) * 128:
                                          (2 * hp + hh + 1) * 128, :]
                                    .rearrange("p (jj t) -> p jj t", jj=NJJ))
                            for jj in range(NJJ):
                                G = (jj + 1 if mask_mode == "causal"
                                     else N_TCHUNK // 2)
                                qs = qTp[:, jj, :, :].rearrange(
                                    "p hh t -> p (hh t)")
                                accT = at_acc.tile([128, 2, SQ_BLK], bf16,
                                                   tag="accT")
                                av = ps_av.tile([128, SQ_BLK], f32, tag="av")
                                prev = None
                                for g in range(G):
                                    c0 = 2 * g
                                    sp = ps_s.tile([128, 2, SQ_BLK], f32,
                                                   tag="s")
                                    nc.tensor.matmul(
                                        sp[:, 0, :],
                                        kT[:, c0 * 128:(c0 + 1) * 128],
                                        qs, start=True, stop=True)
                                    nc.tensor.matmul(
                                        sp[:, 1, :],
                                        kT[:, (c0 + 1) * 128:(c0 + 2) * 128],
                                        qs, start=True, stop=True)
                                    sp_flat = sp[:].rearrange(
                                        "p a q -> p (a q)")
                                    if mask_mode == "general":
                                        mt = at_mt.tile([128, 2, 2, A2A_TOK],
                                                        f32, tag="mt")
                                        for hh in range(2):
                                            nc.sync.dma_start(
                                                mt[:, :, hh, :],
                                                maskT.ap()[
                                                    c0 * 128:(c0 + 2) * 128,
                                                    jj * A2A_TOK:
                                                    (jj + 1) * A2A_TOK]
                                                .rearrange("(c p) q -> p c q",
                                                           p=128))
                                        nc.vector.tensor_add(
                                            sp_flat, sp_flat,
                                            mt[:].rearrange(
                                                "p c hh q -> p (c hh q)"))
                                    e = at_e.tile([128, 2, SQ_BLK], bf16,
                                                  tag="e")
                                    e_flat = e[:].rearrange("p a q -> p (a q)")
                                    nc.scalar.activation(e_flat, sp_flat, Exp,
                                                         scale=SCALE)
                                    if mask_mode == "causal" and g == G - 1:
                                        nc.vector.tensor_mul(
                                            e_flat, e_flat,
                                            cm[:].rearrange(
                                                "p a hh t -> p (a hh t)"))
                                    if g == 0:
                                        nc.vector.tensor_copy(accT[:], e[:])
                                    else:
                                        nc.vector.tensor_add(accT[:], accT[:],
                                                             e[:])
                                    if prev is not None:
                                        pc0, pe = prev
                                        nc.tensor.matmul(
                                            av[:], v_nat[:, pc0, :],
                                            pe[:, 0, :],
                                            start=(pc0 == 0), stop=False)
                                        nc.tensor.matmul(
                                            av[:], v_nat[:, pc0 + 1, :],
                                            pe[:, 1, :],
                                            start=False, stop=False)
                                    prev = (c0, e)
                                pc0, pe = prev
                                nc.tensor.matmul(
                                    av[:], v_nat[:, pc0, :], pe[:, 0, :],
                                    start=(pc0 == 0), stop=False)
                                nc.tensor.matmul(
                                    av[:], v_nat[:, pc0 + 1, :], pe[:, 1, :],
                                    start=False, stop=True)
                                # sender-side softmax normalization
                                sm = ps_sm.tile([1, SQ_BLK], f32, tag="sm")
                                nc.tensor.matmul(sm[:], ones_col[:],
                                                 accT[:, 0, :],
                                                 start=True, stop=False)
                                nc.tensor.matmul(sm[:], ones_col[:],
                                                 accT[:, 1, :],
                                                 start=False, stop=True)
                                rs = at_nrm.tile([1, SQ_BLK], f32, tag="rs")
                                nc.vector.reciprocal_approx_fast(out=rs[:],
                                                                 in_=sm[:])
                                rb = at_nrm.tile([128, SQ_BLK], f32, tag="rb")
                                nc.gpsimd.partition_broadcast(rb[:], rs[:])
                                at = at_o.tile([128, SQ_BLK], bf16, tag="at")
                                nc.vector.tensor_mul(at[:], av[:], rb[:])
                                # tokens [256jj, 256jj+256) of batch b go to
                                # rank jj; rows hh*128.. of the hp payload
                                nc.sync.dma_start(
                                    a2a_in[b][hp][jj].rearrange(
                                        "(hh p) t -> p hh t", p=128),
                                    at[:].rearrange("p (hh t) -> p hh t",
                                                    hh=2))
                            nc.gpsimd.collective_compute(
                                "AllToAll", mybir.AluOpType.bypass,
                                replica_groups=[list(range(N_CORES))],
                                ins=[a2a_in[b][hp].opt()],
                                outs=[a2a_out[b][hp].opt()],
                            )

                # ------------- stage 3: wo projection (2x256 owned tokens) ----
                # k-order visits the hp=0 kv-chunks first so accumulation can
                # start while the second half of the last AllToAll lands.
                korder = [kc for kc in range(NKC) if kc % 4 < 2] + \
                         [kc for kc in range(NKC) if kc % 4 >= 2]
                with tc.tile_pool(name="wo_a", bufs=1) as wo_a, \
                     tc.tile_pool(name="wo_o", bufs=3) as wo_o, \
                     tc.tile_pool(name="wo_ps", bufs=4, space="PSUM") as wo_ps:
                    a_sb = wo_a.tile([128, NKC, SQ_BLK], bf16)
                    for kc in korder:
                        r_, hh = kc // 4, kc % 4
                        for b in range(B):
                            nc.sync.dma_start(
                                a_sb[:, kc, b * A2A_TOK:(b + 1) * A2A_TOK],
                                a2a_out[b][hh // 2][r_,
                                                    (hh % 2) * 128:
                                                    (hh % 2 + 1) * 128, :])
                    for m in range(NKC):
                        w_sb2 = wo_w.tile([128, NKC, 128], bf16, tag="w")
                        nc.sync.dma_start(w_sb2[:], woT4.ap()[m])
                        ps = wo_ps.tile([128, SQ_BLK], f32, tag="ps")
                        for ki, k in enumerate(korder):
                            nc.tensor.matmul(
                                ps[:], w_sb2[:, k, :], a_sb[:, k, :],
                                start=(ki == 0), stop=(ki == NKC - 1))
                        o_sb = wo_o.tile([128, SQ_BLK], bf16, tag="o")
                        nc.vector.tensor_copy(o_sb[:], ps[:])
                        nc.sync.dma_start(
                            out_d.ap()[m * 128:(m + 1) * 128, :], o_sb[:])

    nc.compile()
    return nc


def _get_program(mask_mode):
    if mask_mode not in _PROGRAMS:
        _PROGRAMS[mask_mode] = _build_program(mask_mode)
    return _PROGRAMS[mask_mode]


def _classify_mask(m2):
    if not m2.any():
        return "none"
    causal_ref = np.triu(np.full((S, S), -1e9, dtype=np.float32), k=1)
    return "causal" if np.array_equal(m2, causal_ref) else "general"


def _prep_inputs(x, freqs_cos, freqs_sin, mask, wq, wk, wv, wo):
    """Host-side sharding / layout prep shared by kernel() and test.py."""
    m2 = np.asarray(mask, np.float32).reshape(S, S)
    mask_mode = _classify_mask(m2)

    xT = np.ascontiguousarray(
        np.asarray(x, np.float32).reshape(TOK, D).T).astype(BF16)
    woT = np.asarray(wo, np.float32).T          # [hd_in, D_out]
    # pre-tile wo for contiguous stationary-block DMAs:
    # woT4[m, p, k, mcol] = woT[k*128+p, m*128+mcol]
    woT4 = np.ascontiguousarray(
        woT.reshape(NKC, 128, NKC, 128).transpose(2, 1, 0, 3)).astype(BF16)

    fc = np.asarray(freqs_cos, np.float32)
    fs = np.asarray(freqs_sin, np.float32)
    cos2 = np.ascontiguousarray(fc.T[_I_OF_P, :]).astype(BF16)    # [128, S]
    sgn = np.where(_IS_ODD, 1.0, -1.0).astype(np.float32)[:, None]
    sin2 = np.ascontiguousarray(fs.T[_I_OF_P, :] * sgn).astype(BF16)

    def permute_heads(w):
        w4 = np.asarray(w, np.float32).reshape(-1, HEAD_DIM, D)
        return w4[:, PERM, :].reshape(-1, D)

    wq_p = permute_heads(wq)
    wk_p = permute_heads(wk)
    wv = np.asarray(wv, np.float32)

    in_maps = []
    for c in range(N_CORES):
        wqkvT = np.ascontiguousarray(np.concatenate(
            [wq_p[c * 512:(c + 1) * 512], wk_p[c * 128:(c + 1) * 128],
             wv[c * 128:(c + 1) * 128]], axis=0).T).astype(BF16)   # [D, 768]
        m = {"xT": xT, "wqkvT": wqkvT, "woT4": woT4, "cos2": cos2, "sin2": sin2}
        if mask_mode == "general":
            m["maskT"] = np.ascontiguousarray(m2.T)
        in_maps.append(m)
    return mask_mode, in_maps


def kernel(x, start_pos, freqs_cos, freqs_sin, mask, cache_k, cache_v,
           wq, wk, wv, wo):
    from concourse.bass_utils import run_bass_kernel_spmd

    assert int(start_pos) == 0, "kernel compiled for start_pos == 0"
    mask_mode, in_maps = _prep_inputs(x, freqs_cos, freqs_sin, mask,
                                      wq, wk, wv, wo)
    nc = _get_program(mask_mode)
    res = run_bass_kernel_spmd(nc, in_maps, list(range(N_CORES)))
    out = np.empty((TOK, D), dtype=np.float32)
    for c in range(N_CORES):
        blk = np.asarray(res.results[c]["out"]).astype(np.float32)  # [D, 512]
        for b in range(B):
            rows = slice(b * S + A2A_TOK * c, b * S + A2A_TOK * (c + 1))
            out[rows, :] = blk[:, b * A2A_TOK:(b + 1) * A2A_TOK].T
    return out.reshape(B, S, D)


# revision 17
# speedup vs baseline: 1.0471x; 1.0471x over previous
"""Trainium2 Bass kernel for nn_Attention (llama-style attention block, GQA, RoPE).

v2 — bf16 dataflow (rel-err gate 2e-2; bf16 lands ~1e-3):
  - All matmul operands bf16 (PSUM accumulation stays f32): same PE cycle
    count as f32r but half the HBM/DMA traffic everywhere.
  - Projection runs 1024-token blocks with 1024-wide moving operands
    (PSUM tiles spanning 2 banks), halving PE instruction count.
  - Softmax denominators no longer burn PE matmul cycles per score chunk:
    a bf16 DVE accumulator (4x mode) sums the exp tiles, one tiny
    ones-matmul per q-block reduces it across partitions.
  - Normalization moved to the sender side of the AllToAll (reciprocal on
    DVE, partition_broadcast on Pool) so the wo stage consumes a2a output
    directly - no post-collective normalize pass on the critical path.
  - Causal mask applied multiplicatively to the bf16 exp tiles (DVE 4x)
    instead of f32 adds on PSUM.
  - V transposes via the DMA XBAR (16-bit transpose) instead of PE.
  - The batch-0 AllToAll overlaps batch-1 attention; sender-side
    normalization leaves only the batch-1 collective exposed.

Distribution (8 NeuronCores, Megatron-style tensor parallel over heads):
  - Each core gets 4 Q heads + its matching 1 KV head (wq/wk/wv output-dim
    sharded). Attention computed per-core in a transposed dataflow
    (head_dim on partitions, tokens on the free dim).
  - Per-batch AllToAll reshards the (already normalized) attention output
    token-parallel; each core then runs wo for its 2x256-token block
    against the full wo, so no AllReduce is needed.
"""

import sys

if "/opt/trn_rl_repo" not in sys.path:
    sys.path.insert(0, "/opt/trn_rl_repo")

import numpy as np
import ml_dtypes

BF16 = ml_dtypes.bfloat16

N_CORES = 8
B, S, D = 2, 2048, 4096
N_HEADS = 32
N_KV_HEADS = 8
HEAD_DIM = 128
H_PER_CORE = N_HEADS // N_CORES          # 4 q heads per core
TOK = B * S                              # 4096 flattened tokens
QKV_M = H_PER_CORE * HEAD_DIM + 2 * HEAD_DIM  # 768 projection rows per core
PROJ_TOK = 512                           # token block in the projection stage
SQ_BLK = 512                             # moving width in attention (2 heads x 256)
A2A_TOK = 256                            # tokens per rank per per-batch AllToAll
NJJ = S // A2A_TOK                       # 8 q-blocks of 256 per batch
N_TCHUNK = S // HEAD_DIM                 # 16 key chunks per batch
SCALE = 1.0 / float(np.sqrt(HEAD_DIM))
NKC = D // 128                           # 32 contraction chunks

# partition permutation for RoPE: pair (even, odd) lives 16 partitions apart
# inside a 32-partition quadrant, so the rotation is a single stream_shuffle.
_P = np.arange(128)
_I_OF_P = 16 * (_P // 32) + (_P % 32) % 16          # rope pair index 0..63
_IS_ODD = (_P % 32) >= 16
PERM = (2 * _I_OF_P + _IS_ODD.astype(np.int64)).astype(np.int64)  # orig row in head block
SHUF_MASK = [(i + 16) % 32 for i in range(32)]

_PROGRAMS = {}


def _build_program(mask_mode):
    """Build + compile the SPMD program. mask_mode in {'causal', 'none', 'general'}."""
    import concourse.bass as bass
    import concourse.mybir as mybir
    import concourse.tile as tile
    from concourse import bacc

    f32 = mybir.dt.float32
    bf16 = mybir.dt.bfloat16
    Exp = mybir.ActivationFunctionType.Exp

    nc = bacc.Bacc("TRN2", target_bir_lowering=False, debug=False,
                   num_devices=N_CORES)

    xT = nc.dram_tensor("xT", [D, TOK], bf16, kind="ExternalInput")
    wqkvT = nc.dram_tensor("wqkvT", [D, QKV_M], bf16, kind="ExternalInput")
    # wo pre-tiled on host: [m_chunk, p, k_chunk, m_col] so each stationary
    # column-block DMA reads contiguous lines
    woT4 = nc.dram_tensor("woT4", [NKC, 128, NKC, 128], bf16, kind="ExternalInput")
    cos2 = nc.dram_tensor("cos2", [128, S], bf16, kind="ExternalInput")
    sin2 = nc.dram_tensor("sin2", [128, S], bf16, kind="ExternalInput")
    if mask_mode == "general":
        # additive mask stored transposed: maskT[k_pos, q_pos]
        maskT = nc.dram_tensor("maskT", [S, S], f32, kind="ExternalInput")
    out_d = nc.dram_tensor("out", [D, SQ_BLK], bf16, kind="ExternalOutput")

    xT_t = xT.ap().rearrange("(k p) t -> p k t", p=128)      # [128, 32, TOK]
    wqkvT_t = wqkvT.ap().rearrange("(k p) m -> p k m", p=128)  # [128, 32, 768]

    with tile.TileContext(nc) as tc:
        # at_kt/at_kv/at_q are hoisted alongside the projection pools so the
        # attention preamble DMAs (kT, v transposes, q loads) prefetch during
        # the projection stage instead of serializing at the pool boundary.
        with tc.tile_pool(name="const", bufs=1) as const, \
             tc.tile_pool(name="dram", bufs=1, space="DRAM") as dram, \
             tc.tile_pool(name="at_kt", bufs=2) as at_kt, \
             tc.tile_pool(name="at_kv", bufs=2) as at_kv, \
             tc.tile_pool(name="at_q", bufs=2) as at_q:
            # per-core q/k/v (transposed layout), split per batch
            qd = [dram.tile([H_PER_CORE * 128, S], bf16, name=f"qd{b_}")
                  for b_ in range(B)]
            kd = [dram.tile([128, S], bf16, name=f"kd{b_}") for b_ in range(B)]
            vd = [dram.tile([128, S], bf16, name=f"vd{b_}") for b_ in range(B)]
            a2a_in = [dram.tile([N_CORES, 512, A2A_TOK], bf16,
                                name=f"a2a_in{b_}") for b_ in range(B)]
            a2a_out = [dram.tile([N_CORES, 512, A2A_TOK], bf16,
                                 name=f"a2a_out{b_}") for b_ in range(B)]

            ones_col = const.tile([128, 1], bf16)     # lhsT for column sums
            nc.vector.memset(ones_col[:], 1.0)
            if mask_mode == "causal":
                # multiplicative 0/1 mask for the diagonal chunk-group:
                # cm[p, a, hh, t] = (t - p >= 128*a), same for both packed
                # heads hh
                cm = const.tile([128, 2, 2, A2A_TOK], bf16, name="cm")
                nc.gpsimd.memset(cm[:], 1.0)
                for a in range(2):
                    for hh in range(2):
                        nc.gpsimd.affine_select(
                            out=cm[:, a, hh, :],
                            in_=cm[:, a, hh, :],
                            pattern=[[1, A2A_TOK]], base=-128 * a,
                            channel_multiplier=-1,
                            compare_op=mybir.AluOpType.is_ge, fill=0.0,
                        )

            # ---------------- stage 1: fused QKV projection + RoPE ----------------
            # k-outer / m-inner with 6 live PSUM accumulation groups, so the
            # two half-K x tiles (xA, xB) double-buffer against each other.
            n_blk = TOK // PROJ_TOK   # 8
            HK = NKC // 2
            with tc.tile_pool(name="pj_w", bufs=1) as pj_w, \
                 tc.tile_pool(name="pj_x", bufs=2) as pj_x, \
                 tc.tile_pool(name="pj_cs", bufs=2) as pj_cs, \
                 tc.tile_pool(name="pj_t", bufs=2) as pj_t, \
                 tc.tile_pool(name="pj_o", bufs=2) as pj_o, \
                 tc.tile_pool(name="pj_ps", bufs=8, space="PSUM") as pj_ps:
                w_sb = pj_w.tile([128, NKC, QKV_M], bf16)
                x0 = slice(0, PROJ_TOK)
                xA0 = pj_x.tile([128, HK, PROJ_TOK], bf16, tag="xA")
                xB0 = pj_x.tile([128, HK, PROJ_TOK], bf16, tag="xB")
                # first x tile split by k-chunk so matmuls start immediately
                for kw in range(0, HK, 4):
                    nc.sync.dma_start(xA0[:, kw:kw + 4, :],
                                      xT_t[:, kw:kw + 4, x0])
                # split the weight load by k-chunk so the first matmuls can
                # start before the full 6.3MB arrives
                for kw in range(0, NKC, 4):
                    nc.sync.dma_start(w_sb[:, kw:kw + 4, :],
                                      wqkvT_t[:, kw:kw + 4, :])
                    if kw == 0:
                        nc.sync.dma_start(xB0[:], xT_t[:, HK:NKC, x0])
                for n in range(n_blk):
                    s0 = (n * PROJ_TOK) % S  # position within the batch
                    bn = n // (S // PROJ_TOK)  # batch of this token block
                    cols = slice(n * PROJ_TOK, (n + 1) * PROJ_TOK)
                    bcols = slice(s0, s0 + PROJ_TOK)
                    if n == 0:
                        xA, xB = xA0, xB0
                    else:
                        xA = pj_x.tile([128, HK, PROJ_TOK], bf16, tag="xA")
                        xB = pj_x.tile([128, HK, PROJ_TOK], bf16, tag="xB")
                        nc.sync.dma_start(xA[:], xT_t[:, 0:HK, cols])
                        nc.sync.dma_start(xB[:], xT_t[:, HK:NKC, cols])
                    c_sb = pj_cs.tile([128, PROJ_TOK], bf16, tag="c")
                    s_sb = pj_cs.tile([128, PROJ_TOK], bf16, tag="s")
                    nc.sync.dma_start(c_sb[:], cos2.ap()[:, s0:s0 + PROJ_TOK])
                    nc.sync.dma_start(s_sb[:], sin2.ap()[:, s0:s0 + PROJ_TOK])
                    pss = [pj_ps.tile([128, PROJ_TOK], f32, tag="ps",
                                      name=f"ps_{n}_{mi}")
                           for mi in range(QKV_M // 128)]
                    for k in range(NKC):
                        xsb = xA if k < HK else xB
                        xi = k if k < HK else k - HK
                        for m in range(QKV_M // 128):
                            nc.tensor.matmul(
                                pss[m][:], w_sb[:, k, m * 128:(m + 1) * 128],
                                xsb[:, xi, :],
                                start=(k == 0), stop=(k == NKC - 1))
                    for m in range(QKV_M // 128):  # q0..q3, k, v
                        ps = pss[m]
                        o_sb = pj_o.tile([128, PROJ_TOK], bf16, tag="o")
                        if m < 5:  # rope for q heads + k
                            tmp = pj_t.tile([128, PROJ_TOK], bf16, tag="tmp")
                            rot = pj_t.tile([128, PROJ_TOK], bf16, tag="rot")
                            t1 = pj_t.tile([128, PROJ_TOK], bf16, tag="t1")
                            nc.scalar.copy(tmp[:], ps[:])
                            nc.vector.stream_shuffle(rot[:], tmp[:], SHUF_MASK)
                            nc.vector.tensor_mul(t1[:], tmp[:], c_sb[:])
                            nc.vector.tensor_mul(rot[:], rot[:], s_sb[:])
                            nc.vector.tensor_add(o_sb[:], t1[:], rot[:])
                        else:
                            nc.scalar.copy(o_sb[:], ps[:])
                        if m < 4:
                            dst = qd[bn][m * 128:(m + 1) * 128, bcols]
                        elif m == 4:
                            dst = kd[bn][:, bcols]
                        else:
                            dst = vd[bn][:, bcols]
                        nc.sync.dma_start(dst, o_sb[:])

            # ---------------- stage 2: attention + per-batch AllToAll --------
            # Two q-heads packed side by side in the 512-wide moving operand
            # (2 x 256 tokens); causality handled at 256-token granularity.
            # wo_w is opened before the attention pools so the wo weight
            # prefetch streams during attention / the collectives. The batch-0
            # collective overlaps batch-1 attention; Pool-stream ordering is
            # kept safe by deferring each hp's normalization broadcast to its
            # end (so it is never queued behind a collective it doesn't need).
            with tc.tile_pool(name="wo_w", bufs=4) as wo_w:
                with tc.tile_pool(name="at_e", bufs=6) as at_e, \
                     tc.tile_pool(name="at_acc", bufs=2) as at_acc, \
                     tc.tile_pool(name="at_nrm", bufs=2) as at_nrm, \
                     tc.tile_pool(name="at_o", bufs=3) as at_o, \
                     tc.tile_pool(name="at_mt", bufs=4) as at_mt, \
                     tc.tile_pool(name="ps_s", bufs=2, space="PSUM") as ps_s, \
                     tc.tile_pool(name="ps_av", bufs=2, space="PSUM") as ps_av, \
                     tc.tile_pool(name="ps_sm", bufs=2, space="PSUM") as ps_sm:
                    for b in range(B):
                        kT = at_kt.tile([128, S], bf16, tag="kT")
                        nc.sync.dma_start(kT[:], kd[b][:])
                        v_nat = at_kv.tile([128, N_TCHUNK, 128], bf16, tag="vn")
                        for i in range(N_TCHUNK):
                            nc.sync.dma_start(v_nat[:, i, :],
                                              vd[b][:, i * 128:(i + 1) * 128],
                                              transpose=True)
                        for hp in range(2):
                            # q for heads (2hp, 2hp+1), interleaved as
                            # [p, jj, hh, t] so each jj reads a contiguous
                            # [128, 512] block covering both heads
                            qTp = at_q.tile([128, NJJ, 2, A2A_TOK], bf16,
                                            tag="qTp")
                            for hh in range(2):
                                nc.sync.dma_start(
                                    qTp[:, :, hh, :],
                                    qd[b][(2 * hp + hh) * 128:
                                          (2 * hp + hh + 1) * 128, :]
                                    .rearrange("p (jj t) -> p jj t", jj=NJJ))
                            # raw (unnormalized) attention outputs and
                            # per-token reciprocal sums for the whole hp;
                            # normalization happens once at hp end so the
                            # Pool broadcast never interleaves with (and gets
                            # blocked by) a collective.
                            avr = at_o.tile([128, NJJ, SQ_BLK], bf16,
                                            tag="avr")
                            rss = at_nrm.tile([1, NJJ, SQ_BLK], f32,
                                              tag="rss")

                            # pending av pair, carried across q-block (jj)
                            # boundaries so the PE stream never drains:
                            # (av_tile, c0, e_tile, is_last, accT, jj)
                            prev = [None]

                            def emit_av(nxt=None):
                                if prev[0] is None:
                                    prev[0] = nxt
                                    return
                                pav, pc0, pe, plast, pacc, pjj = prev[0]
                                nc.tensor.matmul(
                                    pav[:], v_nat[:, pc0, :], pe[:, 0, :],
                                    start=(pc0 == 0), stop=False)
                                nc.tensor.matmul(
                                    pav[:], v_nat[:, pc0 + 1, :], pe[:, 1, :],
                                    start=False, stop=plast)
                                if plast:
                                    sm = ps_sm.tile([1, SQ_BLK], f32,
                                                    tag="sm", name="sm")
                                    nc.tensor.matmul(sm[:], ones_col[:],
                                                     pacc[:, 0, :],
                                                     start=True, stop=False)
                                    nc.tensor.matmul(sm[:], ones_col[:],
                                                     pacc[:, 1, :],
                                                     start=False, stop=True)
                                    nc.vector.reciprocal_approx_fast(
                                        out=rss[:, pjj, :], in_=sm[:])
                                    nc.vector.tensor_copy(avr[:, pjj, :],
                                                          pav[:])
                                prev[0] = nxt

                            for jj in range(NJJ):
                                G = (jj + 1 if mask_mode == "causal"
                                     else N_TCHUNK // 2)
                                qs = qTp[:, jj, :, :].rearrange(
                                    "p hh t -> p (hh t)")
                                accT = at_acc.tile([128, 2, SQ_BLK], bf16,
                                                   tag="accT")
                                av = ps_av.tile([128, SQ_BLK], f32, tag="av")
                                for g in range(G):
                                    c0 = 2 * g
                                    sp = ps_s.tile([128, 2, SQ_BLK], f32,
                                                   tag="s")
                                    nc.tensor.matmul(
                                        sp[:, 0, :],
                                        kT[:, c0 * 128:(c0 + 1) * 128],
                                        qs, start=True, stop=True)
                                    nc.tensor.matmul(
                                        sp[:, 1, :],
                                        kT[:, (c0 + 1) * 128:(c0 + 2) * 128],
                                        qs, start=True, stop=True)
                                    sp_flat = sp[:].rearrange(
                                        "p a q -> p (a q)")
                                    if mask_mode == "general":
                                        mt = at_mt.tile([128, 2, 2, A2A_TOK],
                                                        f32, tag="mt")
                                        for hh in range(2):
                                            nc.sync.dma_start(
                                                mt[:, :, hh, :],
                                                maskT.ap()[
                                                    c0 * 128:(c0 + 2) * 128,
                                                    jj * A2A_TOK:
                                                    (jj + 1) * A2A_TOK]
                                                .rearrange("(c p) q -> p c q",
                                                           p=128))
                                        nc.vector.tensor_add(
                                            sp_flat, sp_flat,
                                            mt[:].rearrange(
                                                "p c hh q -> p (c hh q)"))
                                    # first group's exp writes the
                                    # accumulator directly
                                    e = (accT if g == 0 else
                                         at_e.tile([128, 2, SQ_BLK], bf16,
                                                   tag="e"))
                                    e_flat = e[:].rearrange("p a q -> p (a q)")
                                    nc.scalar.activation(e_flat, sp_flat, Exp,
                                                         scale=SCALE)
                                    if mask_mode == "causal" and g == G - 1:
                                        nc.vector.tensor_mul(
                                            e_flat, e_flat,
                                            cm[:].rearrange(
                                                "p a hh t -> p (a hh t)"))
                                    # flush the pending av pair BEFORE the
                                    # accumulator update: when g==1 the
                                    # pending e aliases accT and must be
                                    # consumed before accT += e
                                    emit_av((av, c0, e, g == G - 1,
                                             accT, jj))
                                    if g > 0:
                                        nc.vector.tensor_add(accT[:], accT[:],
                                                             e[:])
                            emit_av()
                            # one broadcast for all 8 q-blocks, then
                            # normalize + ship the a2a payload
                            rbb = at_nrm.tile([128, NJJ, SQ_BLK], f32,
                                              tag="rbb")
                            nc.gpsimd.partition_broadcast(
                                rbb[:].rearrange("p jj q -> p (jj q)"),
                                rss[:].rearrange("one jj q -> one (jj q)"))
                            for jj in range(NJJ):
                                at = at_o.tile([128, SQ_BLK], bf16, tag="at")
                                nc.vector.tensor_mul(at[:], avr[:, jj, :],
                                                     rbb[:, jj, :])
                                # tokens [256jj, 256jj+256) of batch b go to
                                # rank jj; head rows (2hp+hh)*128
                                nc.sync.dma_start(
                                    a2a_in[b][jj,
                                              2 * hp * 128:
                                              (2 * hp + 2) * 128, :]
                                    .rearrange("(hh p) t -> p hh t", p=128),
                                    at[:].rearrange("p (hh t) -> p hh t",
                                                    hh=2))
                        nc.gpsimd.collective_compute(
                            "AllToAll", mybir.AluOpType.bypass,
                            replica_groups=[list(range(N_CORES))],
                            ins=[a2a_in[b].opt()], outs=[a2a_out[b].opt()],
                        )

                # ------------- stage 3: wo projection (2x256 owned tokens) ----
                with tc.tile_pool(name="wo_a", bufs=1) as wo_a, \
                     tc.tile_pool(name="wo_o", bufs=3) as wo_o, \
                     tc.tile_pool(name="wo_ps", bufs=4, space="PSUM") as wo_ps:
                    a_sb = wo_a.tile([128, NKC, SQ_BLK], bf16)
                    for kc in range(NKC):
                        r_, hh = kc // 4, kc % 4
                        for b in range(B):
                            nc.sync.dma_start(
                                a_sb[:, kc, b * A2A_TOK:(b + 1) * A2A_TOK],
                                a2a_out[b][r_, hh * 128:(hh + 1) * 128, :])
                    for m in range(NKC):
                        w_sb2 = wo_w.tile([128, NKC, 128], bf16, tag="w")
                        nc.sync.dma_start(w_sb2[:], woT4.ap()[m])
                        ps = wo_ps.tile([128, SQ_BLK], f32, tag="ps")
                        for k in range(NKC):
                            nc.tensor.matmul(
                                ps[:], w_sb2[:, k, :], a_sb[:, k, :],
                                start=(k == 0), stop=(k == NKC - 1))
                        o_sb = wo_o.tile([128, SQ_BLK], bf16, tag="o")
                        nc.vector.tensor_copy(o_sb[:], ps[:])
                        nc.sync.dma_start(
                            out_d.ap()[m * 128:(m + 1) * 128, :], o_sb[:])

    nc.compile()
    return nc


def _get_program(mask_mode):
    if mask_mode not in _PROGRAMS:
        _PROGRAMS[mask_mode] = _build_program(mask_mode)
    return _PROGRAMS[mask_mode]


def _classify_mask(m2):
    if not m2.any():
        return "none"
    causal_ref = np.triu(np.full((S, S), -1e9, dtype=np.float32), k=1)
    return "causal" if np.array_equal(m2, causal_ref) else "general"


def _prep_inputs(x, freqs_cos, freqs_sin, mask, wq, wk, wv, wo):
    """Host-side sharding / layout prep shared by kernel() and test.py."""
    m2 = np.asarray(mask, np.float32).reshape(S, S)
    mask_mode = _classify_mask(m2)

    xT = np.ascontiguousarray(
        np.asarray(x, np.float32).reshape(TOK, D).T).astype(BF16)
    woT = np.asarray(wo, np.float32).T          # [hd_in, D_out]
    # pre-tile wo for contiguous stationary-block DMAs:
    # woT4[m, p, k, mcol] = woT[k*128+p, m*128+mcol]
    woT4 = np.ascontiguousarray(
        woT.reshape(NKC, 128, NKC, 128).transpose(2, 1, 0, 3)).astype(BF16)

    fc = np.asarray(freqs_cos, np.float32)
    fs = np.asarray(freqs_sin, np.float32)
    cos2 = np.ascontiguousarray(fc.T[_I_OF_P, :]).astype(BF16)    # [128, S]
    sgn = np.where(_IS_ODD, 1.0, -1.0).astype(np.float32)[:, None]
    sin2 = np.ascontiguousarray(fs.T[_I_OF_P, :] * sgn).astype(BF16)

    def permute_heads(w):
        w4 = np.asarray(w, np.float32).reshape(-1, HEAD_DIM, D)
        return w4[:, PERM, :].reshape(-1, D)

    wq_p = permute_heads(wq)
    wk_p = permute_heads(wk)
    wv = np.asarray(wv, np.float32)

    in_maps = []
    for c in range(N_CORES):
        wqkvT = np.ascontiguousarray(np.concatenate(
            [wq_p[c * 512:(c + 1) * 512], wk_p[c * 128:(c + 1) * 128],
             wv[c * 128:(c + 1) * 128]], axis=0).T).astype(BF16)   # [D, 768]
        m = {"xT": xT, "wqkvT": wqkvT, "woT4": woT4, "cos2": cos2, "sin2": sin2}
        if mask_mode == "general":
            m["maskT"] = np.ascontiguousarray(m2.T)
        in_maps.append(m)
    return mask_mode, in_maps


def kernel(x, start_pos, freqs_cos, freqs_sin, mask, cache_k, cache_v,
           wq, wk, wv, wo):
    from concourse.bass_utils import run_bass_kernel_spmd

    assert int(start_pos) == 0, "kernel compiled for start_pos == 0"
    mask_mode, in_maps = _prep_inputs(x, freqs_cos, freqs_sin, mask,
                                      wq, wk, wv, wo)
    nc = _get_program(mask_mode)
    res = run_bass_kernel_spmd(nc, in_maps, list(range(N_CORES)))
    out = np.empty((TOK, D), dtype=np.float32)
    for c in range(N_CORES):
        blk = np.asarray(res.results[c]["out"]).astype(np.float32)  # [D, 512]
        for b in range(B):
            rows = slice(b * S + A2A_TOK * c, b * S + A2A_TOK * (c + 1))
            out[rows, :] = blk[:, b * A2A_TOK:(b + 1) * A2A_TOK].T
    return out.reshape(B, S, D)


# revision 21
# speedup vs baseline: 1.0804x; 1.0318x over previous
"""Trainium2 Bass kernel for nn_Attention (llama-style attention block, GQA, RoPE).

v2 — bf16 dataflow (rel-err gate 2e-2; bf16 lands ~1e-3):
  - All matmul operands bf16 (PSUM accumulation stays f32): same PE cycle
    count as f32r but half the HBM/DMA traffic everywhere.
  - Projection runs 1024-token blocks with 1024-wide moving operands
    (PSUM tiles spanning 2 banks), halving PE instruction count.
  - Softmax denominators no longer burn PE matmul cycles per score chunk:
    a bf16 DVE accumulator (4x mode) sums the exp tiles, one tiny
    ones-matmul per q-block reduces it across partitions.
  - Normalization moved to the sender side of the AllToAll (reciprocal on
    DVE, partition_broadcast on Pool) so the wo stage consumes a2a output
    directly - no post-collective normalize pass on the critical path.
  - Causal mask applied multiplicatively to the bf16 exp tiles (DVE 4x)
    instead of f32 adds on PSUM.
  - V transposes via the DMA XBAR (16-bit transpose) instead of PE.
  - The batch-0 AllToAll overlaps batch-1 attention; sender-side
    normalization leaves only the batch-1 collective exposed.

Distribution (8 NeuronCores, Megatron-style tensor parallel over heads):
  - Each core gets 4 Q heads + its matching 1 KV head (wq/wk/wv output-dim
    sharded). Attention computed per-core in a transposed dataflow
    (head_dim on partitions, tokens on the free dim).
  - Per-batch AllToAll reshards the (already normalized) attention output
    token-parallel; each core then runs wo for its 2x256-token block
    against the full wo, so no AllReduce is needed.
"""

import sys

if "/opt/trn_rl_repo" not in sys.path:
    sys.path.insert(0, "/opt/trn_rl_repo")

import numpy as np
import ml_dtypes

BF16 = ml_dtypes.bfloat16

N_CORES = 8
B, S, D = 2, 2048, 4096
N_HEADS = 32
N_KV_HEADS = 8
HEAD_DIM = 128
H_PER_CORE = N_HEADS // N_CORES          # 4 q heads per core
TOK = B * S                              # 4096 flattened tokens
QKV_M = H_PER_CORE * HEAD_DIM + 2 * HEAD_DIM  # 768 projection rows per core
PROJ_TOK = 512                           # token block in the projection stage
SQ_BLK = 512                             # moving width in attention (2 heads x 256)
A2A_TOK = 256                            # tokens per rank per per-batch AllToAll
NJJ = S // A2A_TOK                       # 8 q-blocks of 256 per batch
N_TCHUNK = S // HEAD_DIM                 # 16 key chunks per batch
SCALE = 1.0 / float(np.sqrt(HEAD_DIM))
NKC = D // 128                           # 32 contraction chunks

# partition permutation for RoPE: pair (even, odd) lives 16 partitions apart
# inside a 32-partition quadrant, so the rotation is a single stream_shuffle.
_P = np.arange(128)
_I_OF_P = 16 * (_P // 32) + (_P % 32) % 16          # rope pair index 0..63
_IS_ODD = (_P % 32) >= 16
PERM = (2 * _I_OF_P + _IS_ODD.astype(np.int64)).astype(np.int64)  # orig row in head block
SHUF_MASK = [(i + 16) % 32 for i in range(32)]

_PROGRAMS = {}


def _build_program(mask_mode):
    """Build + compile the SPMD program. mask_mode in {'causal', 'none', 'general'}."""
    import concourse.bass as bass
    import concourse.mybir as mybir
    import concourse.tile as tile
    from concourse import bacc

    f32 = mybir.dt.float32
    bf16 = mybir.dt.bfloat16
    Exp = mybir.ActivationFunctionType.Exp

    nc = bacc.Bacc("TRN2", target_bir_lowering=False, debug=False,
                   num_devices=N_CORES)

    xT = nc.dram_tensor("xT", [D, TOK], bf16, kind="ExternalInput")
    wqkvT = nc.dram_tensor("wqkvT", [D, QKV_M], bf16, kind="ExternalInput")
    # wo pre-tiled on host: [m_chunk, p, k_chunk, m_col] so each stationary
    # column-block DMA reads contiguous lines
    woT4 = nc.dram_tensor("woT4", [NKC, 128, NKC, 128], bf16, kind="ExternalInput")
    cos2 = nc.dram_tensor("cos2", [128, S], bf16, kind="ExternalInput")
    sin2 = nc.dram_tensor("sin2", [128, S], bf16, kind="ExternalInput")
    if mask_mode == "general":
        # additive mask stored transposed: maskT[k_pos, q_pos]
        maskT = nc.dram_tensor("maskT", [S, S], f32, kind="ExternalInput")
    out_d = nc.dram_tensor("out", [D, SQ_BLK], bf16, kind="ExternalOutput")

    xT_t = xT.ap().rearrange("(k p) t -> p k t", p=128)      # [128, 32, TOK]
    wqkvT_t = wqkvT.ap().rearrange("(k p) m -> p k m", p=128)  # [128, 32, 768]

    with tile.TileContext(nc) as tc:
        # at_kt/at_kv/at_q are hoisted alongside the projection pools so the
        # attention preamble DMAs (kT, v transposes, q loads) prefetch during
        # the projection stage instead of serializing at the pool boundary.
        with tc.tile_pool(name="const", bufs=1) as const, \
             tc.tile_pool(name="dram", bufs=1, space="DRAM") as dram, \
             tc.tile_pool(name="at_kt", bufs=2) as at_kt, \
             tc.tile_pool(name="at_kv", bufs=2) as at_kv, \
             tc.tile_pool(name="at_q", bufs=2) as at_q:
            # per-core q/k/v (transposed layout), split per batch
            qd = [dram.tile([H_PER_CORE * 128, S], bf16, name=f"qd{b_}")
                  for b_ in range(B)]
            kd = [dram.tile([128, S], bf16, name=f"kd{b_}") for b_ in range(B)]
            vd = [dram.tile([128, S], bf16, name=f"vd{b_}") for b_ in range(B)]
            a2a_in = [dram.tile([N_CORES, 512, A2A_TOK], bf16,
                                name=f"a2a_in{b_}") for b_ in range(B)]
            a2a_out = [dram.tile([N_CORES, 512, A2A_TOK], bf16,
                                 name=f"a2a_out{b_}") for b_ in range(B)]

            ones_col = const.tile([128, 1], bf16)     # lhsT for column sums
            nc.vector.memset(ones_col[:], 1.0)
            if mask_mode == "causal":
                # multiplicative 0/1 mask for the diagonal chunk-group:
                # cm[p, a, hh, t] = (t - p >= 128*a), same for both packed
                # heads hh
                cm = const.tile([128, 2, 2, A2A_TOK], bf16, name="cm")
                nc.gpsimd.memset(cm[:], 1.0)
                for a in range(2):
                    for hh in range(2):
                        nc.gpsimd.affine_select(
                            out=cm[:, a, hh, :],
                            in_=cm[:, a, hh, :],
                            pattern=[[1, A2A_TOK]], base=-128 * a,
                            channel_multiplier=-1,
                            compare_op=mybir.AluOpType.is_ge, fill=0.0,
                        )

            def emit_preamble(b):
                """kT / transposed-v / packed-q loads for one batch's
                attention. Called mid-projection for batch 0 so the DMAs
                overlap the remaining projection blocks."""
                kT = at_kt.tile([128, S], bf16, tag="kT", name=f"kT{b}")
                nc.sync.dma_start(kT[:], kd[b][:])
                v_nat = at_kv.tile([128, N_TCHUNK, 128], bf16, tag="vn",
                                   name=f"vn{b}")
                for i in range(N_TCHUNK):
                    nc.sync.dma_start(v_nat[:, i, :],
                                      vd[b][:, i * 128:(i + 1) * 128],
                                      transpose=True)
                qTps = []
                for hp in range(2):
                    qTp = at_q.tile([128, NJJ, 2, A2A_TOK], bf16, tag="qTp",
                                    name=f"qTp{b}_{hp}")
                    for hh in range(2):
                        nc.sync.dma_start(
                            qTp[:, :, hh, :],
                            qd[b][(2 * hp + hh) * 128:
                                  (2 * hp + hh + 1) * 128, :]
                            .rearrange("p (jj t) -> p jj t", jj=NJJ))
                    qTps.append(qTp)
                return kT, v_nat, qTps

            pre = {}

            # ---------------- stage 1: fused QKV projection + RoPE ----------------
            # k-outer / m-inner with 6 live PSUM accumulation groups, so the
            # two half-K x tiles (xA, xB) double-buffer against each other.
            n_blk = TOK // PROJ_TOK   # 8
            HK = NKC // 2
            with tc.tile_pool(name="pj_w", bufs=1) as pj_w, \
                 tc.tile_pool(name="pj_x", bufs=2) as pj_x, \
                 tc.tile_pool(name="pj_cs", bufs=2) as pj_cs, \
                 tc.tile_pool(name="pj_t", bufs=2) as pj_t, \
                 tc.tile_pool(name="pj_o", bufs=2) as pj_o, \
                 tc.tile_pool(name="pj_ps", bufs=8, space="PSUM") as pj_ps:
                w_sb = pj_w.tile([128, NKC, QKV_M], bf16)
                x0 = slice(0, PROJ_TOK)
                xA0 = pj_x.tile([128, HK, PROJ_TOK], bf16, tag="xA")
                xB0 = pj_x.tile([128, HK, PROJ_TOK], bf16, tag="xB")
                # first x tile split by k-chunk so matmuls start immediately
                for kw in range(0, HK, 4):
                    nc.sync.dma_start(xA0[:, kw:kw + 4, :],
                                      xT_t[:, kw:kw + 4, x0])
                # split the weight load by k-chunk so the first matmuls can
                # start before the full 6.3MB arrives
                for kw in range(0, NKC, 4):
                    nc.sync.dma_start(w_sb[:, kw:kw + 4, :],
                                      wqkvT_t[:, kw:kw + 4, :])
                    if kw == 0:
                        nc.sync.dma_start(xB0[:], xT_t[:, HK:NKC, x0])
                for n in range(n_blk):
                    s0 = (n * PROJ_TOK) % S  # position within the batch
                    bn = n // (S // PROJ_TOK)  # batch of this token block
                    cols = slice(n * PROJ_TOK, (n + 1) * PROJ_TOK)
                    bcols = slice(s0, s0 + PROJ_TOK)
                    if n == 0:
                        xA, xB = xA0, xB0
                    else:
                        xA = pj_x.tile([128, HK, PROJ_TOK], bf16, tag="xA")
                        xB = pj_x.tile([128, HK, PROJ_TOK], bf16, tag="xB")
                        nc.sync.dma_start(xA[:], xT_t[:, 0:HK, cols])
                        nc.sync.dma_start(xB[:], xT_t[:, HK:NKC, cols])
                    c_sb = pj_cs.tile([128, PROJ_TOK], bf16, tag="c")
                    s_sb = pj_cs.tile([128, PROJ_TOK], bf16, tag="s")
                    nc.sync.dma_start(c_sb[:], cos2.ap()[:, s0:s0 + PROJ_TOK])
                    nc.sync.dma_start(s_sb[:], sin2.ap()[:, s0:s0 + PROJ_TOK])
                    pss = [pj_ps.tile([128, PROJ_TOK], f32, tag="ps",
                                      name=f"ps_{n}_{mi}")
                           for mi in range(QKV_M // 128)]
                    for k in range(NKC):
                        xsb = xA if k < HK else xB
                        xi = k if k < HK else k - HK
                        for m in range(QKV_M // 128):
                            nc.tensor.matmul(
                                pss[m][:], w_sb[:, k, m * 128:(m + 1) * 128],
                                xsb[:, xi, :],
                                start=(k == 0), stop=(k == NKC - 1))
                    for m in range(QKV_M // 128):  # q0..q3, k, v
                        ps = pss[m]
                        o_sb = pj_o.tile([128, PROJ_TOK], bf16, tag="o")
                        if m < 5:  # rope for q heads + k
                            tmp = pj_t.tile([128, PROJ_TOK], bf16, tag="tmp")
                            rot = pj_t.tile([128, PROJ_TOK], bf16, tag="rot")
                            t1 = pj_t.tile([128, PROJ_TOK], bf16, tag="t1")
                            nc.scalar.copy(tmp[:], ps[:])
                            nc.vector.stream_shuffle(rot[:], tmp[:], SHUF_MASK)
                            nc.vector.tensor_mul(t1[:], tmp[:], c_sb[:])
                            nc.vector.tensor_mul(rot[:], rot[:], s_sb[:])
                            nc.vector.tensor_add(o_sb[:], t1[:], rot[:])
                        else:
                            nc.scalar.copy(o_sb[:], ps[:])
                        if m < 4:
                            dst = qd[bn][m * 128:(m + 1) * 128, bcols]
                        elif m == 4:
                            dst = kd[bn][:, bcols]
                        else:
                            dst = vd[bn][:, bcols]
                        nc.sync.dma_start(dst, o_sb[:])
                    if n == S // PROJ_TOK - 1:
                        # emit batch-0 attention preamble DMAs here so they
                        # queue ahead of the remaining projection stores and
                        # run as soon as batch 0's q/k/v land in DRAM
                        pre[0] = emit_preamble(0)

            # ---------------- stage 2: attention + per-batch AllToAll --------
            # Two q-heads packed side by side in the 512-wide moving operand
            # (2 x 256 tokens); causality handled at 256-token granularity.
            # wo_w is opened before the attention pools so the wo weight
            # prefetch streams during attention / the collectives. The batch-0
            # collective overlaps batch-1 attention; Pool-stream ordering is
            # kept safe by deferring each hp's normalization broadcast to its
            # end (so it is never queued behind a collective it doesn't need).
            with tc.tile_pool(name="wo_w", bufs=4) as wo_w:
                with tc.tile_pool(name="at_e", bufs=6) as at_e, \
                     tc.tile_pool(name="at_acc", bufs=2) as at_acc, \
                     tc.tile_pool(name="at_nrm", bufs=2) as at_nrm, \
                     tc.tile_pool(name="at_o", bufs=3) as at_o, \
                     tc.tile_pool(name="at_mt", bufs=4) as at_mt, \
                     tc.tile_pool(name="ps_s", bufs=2, space="PSUM") as ps_s, \
                     tc.tile_pool(name="ps_av", bufs=2, space="PSUM") as ps_av, \
                     tc.tile_pool(name="ps_sm", bufs=2, space="PSUM") as ps_sm:
                    for b in range(B):
                        if b not in pre:
                            pre[b] = emit_preamble(b)
                        kT, v_nat, qTps = pre[b]
                        for hp in range(2):
                            qTp = qTps[hp]

                            # pending av pair, carried across q-block (jj)
                            # boundaries so the PE stream never drains:
                            # (av_tile, c0, e_tile, is_last, accT, jj)
                            prev = [None]

                            def emit_av(nxt=None, v_nat=v_nat, b=b, hp=hp):
                                if prev[0] is None:
                                    prev[0] = nxt
                                    return
                                pav, pc0, pe, plast, pacc, pjj = prev[0]
                                nc.tensor.matmul(
                                    pav[:], v_nat[:, pc0, :], pe[:, 0, :],
                                    start=(pc0 == 0), stop=False)
                                nc.tensor.matmul(
                                    pav[:], v_nat[:, pc0 + 1, :], pe[:, 1, :],
                                    start=False, stop=plast)
                                if plast:
                                    # per-jj normalization chain; the avr
                                    # copy releases the av PSUM bank without
                                    # waiting on the Pool broadcast (which
                                    # can queue behind a collective)
                                    avr = at_o.tile([128, SQ_BLK], bf16,
                                                    tag="avr", name="avr")
                                    nc.vector.tensor_copy(avr[:], pav[:])
                                    sm = ps_sm.tile([1, SQ_BLK], f32,
                                                    tag="sm", name="sm")
                                    nc.tensor.matmul(sm[:], ones_col[:],
                                                     pacc[:, 0, :],
                                                     start=True, stop=False)
                                    nc.tensor.matmul(sm[:], ones_col[:],
                                                     pacc[:, 1, :],
                                                     start=False, stop=True)
                                    rs = at_nrm.tile([1, SQ_BLK], f32,
                                                     tag="rs", name="rs")
                                    nc.vector.reciprocal_approx_fast(
                                        out=rs[:], in_=sm[:])
                                    rb = at_nrm.tile([128, SQ_BLK], f32,
                                                     tag="rb", name="rb")
                                    nc.gpsimd.partition_broadcast(rb[:],
                                                                  rs[:])
                                    at = at_o.tile([128, SQ_BLK], bf16,
                                                   tag="at", name="at")
                                    nc.vector.tensor_mul(at[:], avr[:], rb[:])
                                    nc.sync.dma_start(
                                        a2a_in[b][pjj,
                                                  2 * hp * 128:
                                                  (2 * hp + 2) * 128, :]
                                        .rearrange("(hh p) t -> p hh t",
                                                   p=128),
                                        at[:].rearrange("p (hh t) -> p hh t",
                                                        hh=2))
                                prev[0] = nxt

                            for jj in range(NJJ):
                                G = (jj + 1 if mask_mode == "causal"
                                     else N_TCHUNK // 2)
                                qs = qTp[:, jj, :, :].rearrange(
                                    "p hh t -> p (hh t)")
                                accT = at_acc.tile([128, 2, SQ_BLK], bf16,
                                                   tag="accT")
                                av = ps_av.tile([128, SQ_BLK], f32, tag="av")
                                for g in range(G):
                                    c0 = 2 * g
                                    sp = ps_s.tile([128, 2, SQ_BLK], f32,
                                                   tag="s")
                                    nc.tensor.matmul(
                                        sp[:, 0, :],
                                        kT[:, c0 * 128:(c0 + 1) * 128],
                                        qs, start=True, stop=True)
                                    nc.tensor.matmul(
                                        sp[:, 1, :],
                                        kT[:, (c0 + 1) * 128:(c0 + 2) * 128],
                                        qs, start=True, stop=True)
                                    sp_flat = sp[:].rearrange(
                                        "p a q -> p (a q)")
                                    if mask_mode == "general":
                                        mt = at_mt.tile([128, 2, 2, A2A_TOK],
                                                        f32, tag="mt")
                                        for hh in range(2):
                                            nc.sync.dma_start(
                                                mt[:, :, hh, :],
                                                maskT.ap()[
                                                    c0 * 128:(c0 + 2) * 128,
                                                    jj * A2A_TOK:
                                                    (jj + 1) * A2A_TOK]
                                                .rearrange("(c p) q -> p c q",
                                                           p=128))
                                        nc.vector.tensor_add(
                                            sp_flat, sp_flat,
                                            mt[:].rearrange(
                                                "p c hh q -> p (c hh q)"))
                                    # first group's exp writes the
                                    # accumulator directly
                                    e = (accT if g == 0 else
                                         at_e.tile([128, 2, SQ_BLK], bf16,
                                                   tag="e"))
                                    e_flat = e[:].rearrange("p a q -> p (a q)")
                                    nc.scalar.activation(e_flat, sp_flat, Exp,
                                                         scale=SCALE)
                                    if mask_mode == "causal" and g == G - 1:
                                        nc.vector.tensor_mul(
                                            e_flat, e_flat,
                                            cm[:].rearrange(
                                                "p a hh t -> p (a hh t)"))
                                    # flush the pending av pair BEFORE the
                                    # accumulator update: when g==1 the
                                    # pending e aliases accT and must be
                                    # consumed before accT += e
                                    emit_av((av, c0, e, g == G - 1,
                                             accT, jj))
                                    if g > 0:
                                        nc.vector.tensor_add(accT[:], accT[:],
                                                             e[:])
                            emit_av()
                        nc.gpsimd.collective_compute(
                            "AllToAll", mybir.AluOpType.bypass,
                            replica_groups=[list(range(N_CORES))],
                            ins=[a2a_in[b].opt()], outs=[a2a_out[b].opt()],
                        )

                # ------------- stage 3: wo projection (2x256 owned tokens) ----
                with tc.tile_pool(name="wo_a", bufs=1) as wo_a, \
                     tc.tile_pool(name="wo_o", bufs=3) as wo_o, \
                     tc.tile_pool(name="wo_ps", bufs=4, space="PSUM") as wo_ps:
                    a_sb = wo_a.tile([128, NKC, SQ_BLK], bf16)
                    for kc in range(NKC):
                        r_, hh = kc // 4, kc % 4
                        for b in range(B):
                            nc.sync.dma_start(
                                a_sb[:, kc, b * A2A_TOK:(b + 1) * A2A_TOK],
                                a2a_out[b][r_, hh * 128:(hh + 1) * 128, :])
                    for m in range(NKC):
                        w_sb2 = wo_w.tile([128, NKC, 128], bf16, tag="w")
                        nc.sync.dma_start(w_sb2[:], woT4.ap()[m])
                        ps = wo_ps.tile([128, SQ_BLK], f32, tag="ps")
                        for k in range(NKC):
                            nc.tensor.matmul(
                                ps[:], w_sb2[:, k, :], a_sb[:, k, :],
                                start=(k == 0), stop=(k == NKC - 1))
                        o_sb = wo_o.tile([128, SQ_BLK], bf16, tag="o")
                        nc.vector.tensor_copy(o_sb[:], ps[:])
                        nc.sync.dma_start(
                            out_d.ap()[m * 128:(m + 1) * 128, :], o_sb[:])

    nc.compile()
    return nc


def _get_program(mask_mode):
    if mask_mode not in _PROGRAMS:
        _PROGRAMS[mask_mode] = _build_program(mask_mode)
    return _PROGRAMS[mask_mode]


def _classify_mask(m2):
    if not m2.any():
        return "none"
    causal_ref = np.triu(np.full((S, S), -1e9, dtype=np.float32), k=1)
    return "causal" if np.array_equal(m2, causal_ref) else "general"


def _prep_inputs(x, freqs_cos, freqs_sin, mask, wq, wk, wv, wo):
    """Host-side sharding / layout prep shared by kernel() and test.py."""
    m2 = np.asarray(mask, np.float32).reshape(S, S)
    mask_mode = _classify_mask(m2)

    xT = np.ascontiguousarray(
        np.asarray(x, np.float32).reshape(TOK, D).T).astype(BF16)
    woT = np.asarray(wo, np.float32).T          # [hd_in, D_out]
    # pre-tile wo for contiguous stationary-block DMAs:
    # woT4[m, p, k, mcol] = woT[k*128+p, m*128+mcol]
    woT4 = np.ascontiguousarray(
        woT.reshape(NKC, 128, NKC, 128).transpose(2, 1, 0, 3)).astype(BF16)

    fc = np.asarray(freqs_cos, np.float32)
    fs = np.asarray(freqs_sin, np.float32)
    cos2 = np.ascontiguousarray(fc.T[_I_OF_P, :]).astype(BF16)    # [128, S]
    sgn = np.where(_IS_ODD, 1.0, -1.0).astype(np.float32)[:, None]
    sin2 = np.ascontiguousarray(fs.T[_I_OF_P, :] * sgn).astype(BF16)

    def permute_heads(w):
        w4 = np.asarray(w, np.float32).reshape(-1, HEAD_DIM, D)
        return w4[:, PERM, :].reshape(-1, D)

    wq_p = permute_heads(wq)
    wk_p = permute_heads(wk)
    wv = np.asarray(wv, np.float32)

    in_maps = []
    for c in range(N_CORES):
        wqkvT = np.ascontiguousarray(np.concatenate(
            [wq_p[c * 512:(c + 1) * 512], wk_p[c * 128:(c + 1) * 128],
             wv[c * 128:(c + 1) * 128]], axis=0).T).astype(BF16)   # [D, 768]
        m = {"xT": xT, "wqkvT": wqkvT, "woT4": woT4, "cos2": cos2, "sin2": sin2}
        if mask_mode == "general":
            m["maskT"] = np.ascontiguousarray(m2.T)
        in_maps.append(m)
    return mask_mode, in_maps


def kernel(x, start_pos, freqs_cos, freqs_sin, mask, cache_k, cache_v,
           wq, wk, wv, wo):
    from concourse.bass_utils import run_bass_kernel_spmd

    assert int(start_pos) == 0, "kernel compiled for start_pos == 0"
    mask_mode, in_maps = _prep_inputs(x, freqs_cos, freqs_sin, mask,
                                      wq, wk, wv, wo)
    nc = _get_program(mask_mode)
    res = run_bass_kernel_spmd(nc, in_maps, list(range(N_CORES)))
    out = np.empty((TOK, D), dtype=np.float32)
    for c in range(N_CORES):
        blk = np.asarray(res.results[c]["out"]).astype(np.float32)  # [D, 512]
        for b in range(B):
            rows = slice(b * S + A2A_TOK * c, b * S + A2A_TOK * (c + 1))
            out[rows, :] = blk[:, b * A2A_TOK:(b + 1) * A2A_TOK].T
    return out.reshape(B, S, D)


# revision 30
# speedup vs baseline: 1.0907x; 1.0095x over previous
"""Trainium2 Bass kernel for nn_Attention (llama-style attention block, GQA, RoPE).

v2 — bf16 dataflow (rel-err gate 2e-2; bf16 lands ~1e-3):
  - All matmul operands bf16 (PSUM accumulation stays f32): same PE cycle
    count as f32r but half the HBM/DMA traffic everywhere.
  - Projection runs 1024-token blocks with 1024-wide moving operands
    (PSUM tiles spanning 2 banks), halving PE instruction count.
  - Softmax denominators no longer burn PE matmul cycles per score chunk:
    a bf16 DVE accumulator (4x mode) sums the exp tiles, one tiny
    ones-matmul per q-block reduces it across partitions.
  - Normalization moved to the sender side of the AllToAll (reciprocal on
    DVE, partition_broadcast on Pool) so the wo stage consumes a2a output
    directly - no post-collective normalize pass on the critical path.
  - Causal mask applied multiplicatively to the bf16 exp tiles (DVE 4x)
    instead of f32 adds on PSUM.
  - V transposes via the DMA XBAR (16-bit transpose) instead of PE.
  - The batch-0 AllToAll overlaps batch-1 attention; sender-side
    normalization leaves only the batch-1 collective exposed.

Distribution (8 NeuronCores, Megatron-style tensor parallel over heads):
  - Each core gets 4 Q heads + its matching 1 KV head (wq/wk/wv output-dim
    sharded). Attention computed per-core in a transposed dataflow
    (head_dim on partitions, tokens on the free dim).
  - Per-batch AllToAll reshards the (already normalized) attention output
    token-parallel; each core then runs wo for its 2x256-token block
    against the full wo, so no AllReduce is needed.
"""

import sys

if "/opt/trn_rl_repo" not in sys.path:
    sys.path.insert(0, "/opt/trn_rl_repo")

import numpy as np
import ml_dtypes

BF16 = ml_dtypes.bfloat16

N_CORES = 8
B, S, D = 2, 2048, 4096
N_HEADS = 32
N_KV_HEADS = 8
HEAD_DIM = 128
H_PER_CORE = N_HEADS // N_CORES          # 4 q heads per core
TOK = B * S                              # 4096 flattened tokens
QKV_M = H_PER_CORE * HEAD_DIM + 2 * HEAD_DIM  # 768 projection rows per core
PROJ_TOK = 512                           # token block in the projection stage
SQ_BLK = 512                             # moving width in attention (2 heads x 256)
A2A_TOK = 256                            # tokens per rank per per-batch AllToAll
NJJ = S // A2A_TOK                       # 8 q-blocks of 256 per batch
N_TCHUNK = S // HEAD_DIM                 # 16 key chunks per batch
SCALE = 1.0 / float(np.sqrt(HEAD_DIM))
NKC = D // 128                           # 32 contraction chunks

# partition permutation for RoPE: pair (even, odd) lives 16 partitions apart
# inside a 32-partition quadrant, so the rotation is a single stream_shuffle.
_P = np.arange(128)
_I_OF_P = 16 * (_P // 32) + (_P % 32) % 16          # rope pair index 0..63
_IS_ODD = (_P % 32) >= 16
PERM = (2 * _I_OF_P + _IS_ODD.astype(np.int64)).astype(np.int64)  # orig row in head block
SHUF_MASK = [(i + 16) % 32 for i in range(32)]

_PROGRAMS = {}


def _build_program(mask_mode):
    """Build + compile the SPMD program. mask_mode in {'causal', 'none', 'general'}."""
    import concourse.bass as bass
    import concourse.mybir as mybir
    import concourse.tile as tile
    from concourse import bacc
    from concourse.masks import make_identity

    f32 = mybir.dt.float32
    bf16 = mybir.dt.bfloat16
    Exp = mybir.ActivationFunctionType.Exp

    nc = bacc.Bacc("TRN2", target_bir_lowering=False, debug=False,
                   num_devices=N_CORES)

    xT = nc.dram_tensor("xT", [D, TOK], bf16, kind="ExternalInput")
    wqkvT = nc.dram_tensor("wqkvT", [D, QKV_M], bf16, kind="ExternalInput")
    # wo pre-tiled on host: [m_chunk, p, k_chunk, m_col] so each stationary
    # column-block DMA reads contiguous lines
    woT4 = nc.dram_tensor("woT4", [NKC, 128, NKC, 128], bf16, kind="ExternalInput")
    cos2 = nc.dram_tensor("cos2", [128, S], bf16, kind="ExternalInput")
    sin2 = nc.dram_tensor("sin2", [128, S], bf16, kind="ExternalInput")
    if mask_mode == "general":
        # additive mask stored transposed: maskT[k_pos, q_pos]
        maskT = nc.dram_tensor("maskT", [S, S], f32, kind="ExternalInput")
    out_d = nc.dram_tensor("out", [D, SQ_BLK], bf16, kind="ExternalOutput")

    xT_t = xT.ap().rearrange("(k p) t -> p k t", p=128)      # [128, 32, TOK]
    wqkvT_t = wqkvT.ap().rearrange("(k p) m -> p k m", p=128)  # [128, 32, 768]

    with tile.TileContext(nc) as tc:
        # at_kt/at_kv/at_q are hoisted alongside the projection pools so the
        # attention preamble DMAs (kT, v transposes, q loads) prefetch during
        # the projection stage instead of serializing at the pool boundary.
        with tc.tile_pool(name="const", bufs=1) as const, \
             tc.tile_pool(name="dram", bufs=1, space="DRAM") as dram, \
             tc.tile_pool(name="at_kt", bufs=2) as at_kt, \
             tc.tile_pool(name="at_kv", bufs=2) as at_kv, \
             tc.tile_pool(name="at_q", bufs=2) as at_q:
            # per-core q/k/v (transposed layout), split per batch
            qd = [dram.tile([H_PER_CORE * 128, S], bf16, name=f"qd{b_}")
                  for b_ in range(B)]
            kd = [dram.tile([128, S], bf16, name=f"kd{b_}") for b_ in range(B)]
            vd = [dram.tile([128, S], bf16, name=f"vd{b_}") for b_ in range(B)]
            a2a_in = [dram.tile([N_CORES, 512, A2A_TOK], bf16,
                                name=f"a2a_in{b_}") for b_ in range(B)]
            a2a_out = [dram.tile([N_CORES, 512, A2A_TOK], bf16,
                                 name=f"a2a_out{b_}") for b_ in range(B)]

            ones_col = const.tile([128, 1], bf16)     # lhsT for column sums
            nc.vector.memset(ones_col[:], 1.0)
            ident = const.tile([128, 128], bf16)      # PE transpose identity
            make_identity(nc, ident[:])
            if mask_mode == "causal":
                # multiplicative 0/1 mask for the diagonal chunk-group:
                # cm[p, a, hh, t] = (t - p >= 128*a), same for both packed
                # heads hh
                cm = const.tile([128, 2, 2, A2A_TOK], bf16, name="cm")
                nc.gpsimd.memset(cm[:], 1.0)
                for a in range(2):
                    for hh in range(2):
                        nc.gpsimd.affine_select(
                            out=cm[:, a, hh, :],
                            in_=cm[:, a, hh, :],
                            pattern=[[1, A2A_TOK]], base=-128 * a,
                            channel_multiplier=-1,
                            compare_op=mybir.AluOpType.is_ge, fill=0.0,
                        )

            def emit_preamble(b):
                """kT / vT / packed-q loads for one batch's attention.
                Called mid-projection for batch 0 so the DMAs overlap the
                remaining projection blocks."""
                kT = at_kt.tile([128, S], bf16, tag="kT", name=f"kT{b}")
                nc.sync.dma_start(kT[:], kd[b][:])
                vT = at_kv.tile([128, S], bf16, tag="vT", name=f"vT{b}")
                nc.sync.dma_start(vT[:], vd[b][:])
                qTps = []
                for hp in range(2):
                    qTp = at_q.tile([128, NJJ, 2, A2A_TOK], bf16, tag="qTp",
                                    name=f"qTp{b}_{hp}")
                    for hh in range(2):
                        nc.sync.dma_start(
                            qTp[:, :, hh, :],
                            qd[b][(2 * hp + hh) * 128:
                                  (2 * hp + hh + 1) * 128, :]
                            .rearrange("p (jj t) -> p jj t", jj=NJJ))
                    qTps.append(qTp)
                return kT, vT, qTps

            pre = {}

            # ---------------- stage 1: fused QKV projection + RoPE ----------------
            # k-outer / m-inner with 6 live PSUM accumulation groups, so the
            # two half-K x tiles (xA, xB) double-buffer against each other.
            n_blk = TOK // PROJ_TOK   # 8
            HK = NKC // 2
            with tc.tile_pool(name="pj_w", bufs=1) as pj_w, \
                 tc.tile_pool(name="pj_x", bufs=3) as pj_x, \
                 tc.tile_pool(name="pj_cs", bufs=2) as pj_cs, \
                 tc.tile_pool(name="pj_t", bufs=2) as pj_t, \
                 tc.tile_pool(name="pj_o", bufs=2) as pj_o, \
                 tc.tile_pool(name="pj_ps", bufs=8, space="PSUM") as pj_ps:
                w_sb = pj_w.tile([128, NKC, QKV_M], bf16)
                x0 = slice(0, PROJ_TOK)
                xA0 = pj_x.tile([128, HK, PROJ_TOK], bf16, tag="xA")
                xB0 = pj_x.tile([128, HK, PROJ_TOK], bf16, tag="xB")
                # first x tile split by k-chunk so matmuls start immediately
                for kw in range(0, HK, 4):
                    nc.sync.dma_start(xA0[:, kw:kw + 4, :],
                                      xT_t[:, kw:kw + 4, x0])
                # split the weight load by k-chunk so the first matmuls can
                # start before the full 6.3MB arrives
                for kw in range(0, NKC, 4):
                    nc.sync.dma_start(w_sb[:, kw:kw + 4, :],
                                      wqkvT_t[:, kw:kw + 4, :])
                    if kw == 0:
                        nc.sync.dma_start(xB0[:], xT_t[:, HK:NKC, x0])
                for n in range(n_blk):
                    s0 = (n * PROJ_TOK) % S  # position within the batch
                    bn = n // (S // PROJ_TOK)  # batch of this token block
                    cols = slice(n * PROJ_TOK, (n + 1) * PROJ_TOK)
                    bcols = slice(s0, s0 + PROJ_TOK)
                    if n == 0:
                        xA, xB = xA0, xB0
                    else:
                        xA = pj_x.tile([128, HK, PROJ_TOK], bf16, tag="xA")
                        xB = pj_x.tile([128, HK, PROJ_TOK], bf16, tag="xB")
                        nc.sync.dma_start(xA[:], xT_t[:, 0:HK, cols])
                        nc.sync.dma_start(xB[:], xT_t[:, HK:NKC, cols])
                    c_sb = pj_cs.tile([128, PROJ_TOK], bf16, tag="c")
                    s_sb = pj_cs.tile([128, PROJ_TOK], bf16, tag="s")
                    nc.sync.dma_start(c_sb[:], cos2.ap()[:, s0:s0 + PROJ_TOK])
                    nc.sync.dma_start(s_sb[:], sin2.ap()[:, s0:s0 + PROJ_TOK])
                    # m-outer / k-inner: each head's full contraction finishes
                    # first, so its RoPE drain overlaps the next head's
                    # matmuls (keeps the block tail short)
                    for m in range(QKV_M // 128):  # q0..q3, k, v
                        ps = pj_ps.tile([128, PROJ_TOK], f32, tag="ps",
                                        name=f"ps_{n}_{m}")
                        for k in range(NKC):
                            xsb = xA if k < HK else xB
                            xi = k if k < HK else k - HK
                            nc.tensor.matmul(
                                ps[:], w_sb[:, k, m * 128:(m + 1) * 128],
                                xsb[:, xi, :],
                                start=(k == 0), stop=(k == NKC - 1))
                        o_sb = pj_o.tile([128, PROJ_TOK], bf16, tag="o")
                        if m < 5:  # rope for q heads + k
                            tmp = pj_t.tile([128, PROJ_TOK], bf16, tag="tmp")
                            rot = pj_t.tile([128, PROJ_TOK], bf16, tag="rot")
                            t1 = pj_t.tile([128, PROJ_TOK], bf16, tag="t1")
                            nc.scalar.copy(tmp[:], ps[:])
                            nc.vector.stream_shuffle(rot[:], tmp[:], SHUF_MASK)
                            nc.vector.tensor_mul(t1[:], tmp[:], c_sb[:])
                            nc.vector.tensor_mul(rot[:], rot[:], s_sb[:])
                            nc.vector.tensor_add(o_sb[:], t1[:], rot[:])
                        else:
                            nc.scalar.copy(o_sb[:], ps[:])
                        if m < 4:
                            dst = qd[bn][m * 128:(m + 1) * 128, bcols]
                        elif m == 4:
                            dst = kd[bn][:, bcols]
                        else:
                            dst = vd[bn][:, bcols]
                        nc.sync.dma_start(dst, o_sb[:])
                    if n == S // PROJ_TOK - 1:
                        # emit batch-0 attention preamble DMAs here so they
                        # queue ahead of the remaining projection stores and
                        # run as soon as batch 0's q/k/v land in DRAM
                        pre[0] = emit_preamble(0)

            # ---------------- stage 2: attention + per-batch AllToAll --------
            # Two q-heads packed side by side in the 512-wide moving operand
            # (2 x 256 tokens); causality handled at 256-token granularity.
            # wo_w is opened before the attention pools so the wo weight
            # prefetch streams during attention / the collectives. The batch-0
            # collective overlaps batch-1 attention; Pool-stream ordering is
            # kept safe by deferring each hp's normalization broadcast to its
            # end (so it is never queued behind a collective it doesn't need).
            with tc.tile_pool(name="wo_w", bufs=4) as wo_w:
                with tc.tile_pool(name="at_e", bufs=6) as at_e, \
                     tc.tile_pool(name="at_acc", bufs=2) as at_acc, \
                     tc.tile_pool(name="at_nrm", bufs=2) as at_nrm, \
                     tc.tile_pool(name="at_o", bufs=3) as at_o, \
                     tc.tile_pool(name="at_mt", bufs=4) as at_mt, \
                     tc.tile_pool(name="ps_s", bufs=2, space="PSUM") as ps_s, \
                     tc.tile_pool(name="ps_av", bufs=2, space="PSUM") as ps_av, \
                     tc.tile_pool(name="ps_sm", bufs=1, space="PSUM") as ps_sm, \
                     tc.tile_pool(name="ps_tr", bufs=1, space="PSUM") as ps_tr:
                    for b in range(B):
                        if b not in pre:
                            pre[b] = emit_preamble(b)
                        kT, vT, qTps = pre[b]
                        v_nat = at_kv.tile([128, N_TCHUNK, 128], bf16,
                                           tag="vn", name=f"vn{b}")
                        for i in range(N_TCHUNK):
                            tp = ps_tr.tile([128, 128], bf16, tag="tr",
                                            name="tp")
                            nc.tensor.transpose(
                                tp[:], vT[:, i * 128:(i + 1) * 128], ident[:])
                            nc.scalar.copy(v_nat[:, i, :], tp[:])
                        for hp in range(2):
                            qTp = qTps[hp]

                            # pending av pair, carried across q-block (jj)
                            # boundaries so the PE stream never drains:
                            # (av_tile, c0, e_tile, is_last, accT, jj)
                            prev = [None]
                            # pending normalization chain, deferred one more
                            # group so the PE isn't gated on the DVE
                            # accumulator finishing at each jj boundary
                            post = [None]

                            def emit_post(b=b, hp=hp):
                                if post[0] is None:
                                    return
                                pav, pacc, pjj = post[0]
                                # the avr copy releases the av PSUM bank
                                # without waiting on the Pool broadcast
                                # (which can queue behind a collective)
                                avr = at_o.tile([128, SQ_BLK], bf16,
                                                tag="avr", name="avr")
                                nc.vector.tensor_copy(avr[:], pav[:])
                                sm = ps_sm.tile([1, SQ_BLK], f32,
                                                tag="sm", name="sm")
                                nc.tensor.matmul(sm[:], ones_col[:],
                                                 pacc[:, 0, :],
                                                 start=True, stop=False)
                                nc.tensor.matmul(sm[:], ones_col[:],
                                                 pacc[:, 1, :],
                                                 start=False, stop=True)
                                rs = at_nrm.tile([1, SQ_BLK], f32,
                                                 tag="rs", name="rs")
                                nc.vector.reciprocal_approx_fast(
                                    out=rs[:], in_=sm[:])
                                rb = at_nrm.tile([128, SQ_BLK], f32,
                                                 tag="rb", name="rb")
                                nc.gpsimd.partition_broadcast(rb[:], rs[:])
                                at = at_o.tile([128, SQ_BLK], bf16,
                                               tag="at", name="at")
                                nc.vector.tensor_mul(at[:], avr[:], rb[:])
                                nc.sync.dma_start(
                                    a2a_in[b][pjj,
                                              2 * hp * 128:
                                              (2 * hp + 2) * 128, :]
                                    .rearrange("(hh p) t -> p hh t", p=128),
                                    at[:].rearrange("p (hh t) -> p hh t",
                                                    hh=2))
                                post[0] = None

                            def emit_av(nxt=None, v_nat=v_nat):
                                if prev[0] is None:
                                    prev[0] = nxt
                                    return
                                pav, pc0, pe, plast, pacc, pjj = prev[0]
                                nc.tensor.matmul(
                                    pav[:], v_nat[:, pc0, :], pe[:, 0, :],
                                    start=(pc0 == 0), stop=False)
                                nc.tensor.matmul(
                                    pav[:], v_nat[:, pc0 + 1, :], pe[:, 1, :],
                                    start=False, stop=plast)
                                emit_post()
                                if plast:
                                    post[0] = (pav, pacc, pjj)
                                prev[0] = nxt

                            for jj in range(NJJ):
                                G = (jj + 1 if mask_mode == "causal"
                                     else N_TCHUNK // 2)
                                qs = qTp[:, jj, :, :].rearrange(
                                    "p hh t -> p (hh t)")
                                accT = at_acc.tile([128, 2, SQ_BLK], bf16,
                                                   tag="accT")
                                av = ps_av.tile([128, SQ_BLK], f32, tag="av")
                                for g in range(G):
                                    c0 = 2 * g
                                    sp = ps_s.tile([128, 2, SQ_BLK], f32,
                                                   tag="s")
                                    nc.tensor.matmul(
                                        sp[:, 0, :],
                                        kT[:, c0 * 128:(c0 + 1) * 128],
                                        qs, start=True, stop=True)
                                    nc.tensor.matmul(
                                        sp[:, 1, :],
                                        kT[:, (c0 + 1) * 128:(c0 + 2) * 128],
                                        qs, start=True, stop=True)
                                    sp_flat = sp[:].rearrange(
                                        "p a q -> p (a q)")
                                    if mask_mode == "general":
                                        mt = at_mt.tile([128, 2, 2, A2A_TOK],
                                                        f32, tag="mt")
                                        for hh in range(2):
                                            nc.sync.dma_start(
                                                mt[:, :, hh, :],
                                                maskT.ap()[
                                                    c0 * 128:(c0 + 2) * 128,
                                                    jj * A2A_TOK:
                                                    (jj + 1) * A2A_TOK]
                                                .rearrange("(c p) q -> p c q",
                                                           p=128))
                                        nc.vector.tensor_add(
                                            sp_flat, sp_flat,
                                            mt[:].rearrange(
                                                "p c hh q -> p (c hh q)"))
                                    # first group's exp writes the
                                    # accumulator directly
                                    e = (accT if g == 0 else
                                         at_e.tile([128, 2, SQ_BLK], bf16,
                                                   tag="e"))
                                    e_flat = e[:].rearrange("p a q -> p (a q)")
                                    nc.scalar.activation(e_flat, sp_flat, Exp,
                                                         scale=SCALE)
                                    if mask_mode == "causal" and g == G - 1:
                                        nc.vector.tensor_mul(
                                            e_flat, e_flat,
                                            cm[:].rearrange(
                                                "p a hh t -> p (a hh t)"))
                                    # flush the pending av pair BEFORE the
                                    # accumulator update: when g==1 the
                                    # pending e aliases accT and must be
                                    # consumed before accT += e
                                    emit_av((av, c0, e, g == G - 1,
                                             accT, jj))
                                    if g > 0:
                                        nc.vector.tensor_add(accT[:], accT[:],
                                                             e[:])
                            emit_av()
                            emit_post()
                        nc.gpsimd.collective_compute(
                            "AllToAll", mybir.AluOpType.bypass,
                            replica_groups=[list(range(N_CORES))],
                            ins=[a2a_in[b].opt()], outs=[a2a_out[b].opt()],
                        )

                # ------------- stage 3: wo projection (2x256 owned tokens) ----
                with tc.tile_pool(name="wo_a", bufs=1) as wo_a, \
                     tc.tile_pool(name="wo_o", bufs=3) as wo_o, \
                     tc.tile_pool(name="wo_ps", bufs=4, space="PSUM") as wo_ps:
                    a_sb = wo_a.tile([128, NKC, SQ_BLK], bf16)
                    for kc in range(NKC):
                        r_, hh = kc // 4, kc % 4
                        for b in range(B):
                            nc.sync.dma_start(
                                a_sb[:, kc, b * A2A_TOK:(b + 1) * A2A_TOK],
                                a2a_out[b][r_, hh * 128:(hh + 1) * 128, :])
                    for m in range(NKC):
                        w_sb2 = wo_w.tile([128, NKC, 128], bf16, tag="w")
                        nc.sync.dma_start(w_sb2[:], woT4.ap()[m])
                        ps = wo_ps.tile([128, SQ_BLK], f32, tag="ps")
                        for k in range(NKC):
                            nc.tensor.matmul(
                                ps[:], w_sb2[:, k, :], a_sb[:, k, :],
                                start=(k == 0), stop=(k == NKC - 1))
                        o_sb = wo_o.tile([128, SQ_BLK], bf16, tag="o")
                        nc.vector.tensor_copy(o_sb[:], ps[:])
                        nc.sync.dma_start(
                            out_d.ap()[m * 128:(m + 1) * 128, :], o_sb[:])

    nc.compile()
    return nc


def _get_program(mask_mode):
    if mask_mode not in _PROGRAMS:
        _PROGRAMS[mask_mode] = _build_program(mask_mode)
    return _PROGRAMS[mask_mode]


def _classify_mask(m2):
    if not m2.any():
        return "none"
    causal_ref = np.triu(np.full((S, S), -1e9, dtype=np.float32), k=1)
    return "causal" if np.array_equal(m2, causal_ref) else "general"


def _prep_inputs(x, freqs_cos, freqs_sin, mask, wq, wk, wv, wo):
    """Host-side sharding / layout prep shared by kernel() and test.py."""
    m2 = np.asarray(mask, np.float32).reshape(S, S)
    mask_mode = _classify_mask(m2)

    xT = np.ascontiguousarray(
        np.asarray(x, np.float32).reshape(TOK, D).T).astype(BF16)
    woT = np.asarray(wo, np.float32).T          # [hd_in, D_out]
    # pre-tile wo for contiguous stationary-block DMAs:
    # woT4[m, p, k, mcol] = woT[k*128+p, m*128+mcol]
    woT4 = np.ascontiguousarray(
        woT.reshape(NKC, 128, NKC, 128).transpose(2, 1, 0, 3)).astype(BF16)

    fc = np.asarray(freqs_cos, np.float32)
    fs = np.asarray(freqs_sin, np.float32)
    cos2 = np.ascontiguousarray(fc.T[_I_OF_P, :]).astype(BF16)    # [128, S]
    sgn = np.where(_IS_ODD, 1.0, -1.0).astype(np.float32)[:, None]
    sin2 = np.ascontiguousarray(fs.T[_I_OF_P, :] * sgn).astype(BF16)

    def permute_heads(w):
        w4 = np.asarray(w, np.float32).reshape(-1, HEAD_DIM, D)
        return w4[:, PERM, :].reshape(-1, D)

    wq_p = permute_heads(wq)
    wk_p = permute_heads(wk)
    wv = np.asarray(wv, np.float32)

    in_maps = []
    for c in range(N_CORES):
        wqkvT = np.ascontiguousarray(np.concatenate(
            [wq_p[c * 512:(c + 1) * 512], wk_p[c * 128:(c + 1) * 128],
             wv[c * 128:(c + 1) * 128]], axis=0).T).astype(BF16)   # [D, 768]
        m = {"xT": xT, "wqkvT": wqkvT, "woT4": woT4, "cos2": cos2, "sin2": sin2}
        if mask_mode == "general":
            m["maskT"] = np.ascontiguousarray(m2.T)
        in_maps.append(m)
    return mask_mode, in_maps


def kernel(x, start_pos, freqs_cos, freqs_sin, mask, cache_k, cache_v,
           wq, wk, wv, wo):
    from concourse.bass_utils import run_bass_kernel_spmd

    assert int(start_pos) == 0, "kernel compiled for start_pos == 0"
    mask_mode, in_maps = _prep_inputs(x, freqs_cos, freqs_sin, mask,
                                      wq, wk, wv, wo)
    nc = _get_program(mask_mode)
    res = run_bass_kernel_spmd(nc, in_maps, list(range(N_CORES)))
    out = np.empty((TOK, D), dtype=np.float32)
    for c in range(N_CORES):
        blk = np.asarray(res.results[c]["out"]).astype(np.float32)  # [D, 512]
        for b in range(B):
            rows = slice(b * S + A2A_TOK * c, b * S + A2A_TOK * (c + 1))
            out[rows, :] = blk[:, b * A2A_TOK:(b + 1) * A2A_TOK].T
    return out.reshape(B, S, D)


# revision 37
# speedup vs baseline: 1.1206x; 1.0275x over previous
"""Trainium2 Bass kernel for nn_Attention (llama-style attention block, GQA, RoPE).

v2 — bf16 dataflow (rel-err gate 2e-2; bf16 lands ~1e-3):
  - All matmul operands bf16 (PSUM accumulation stays f32): same PE cycle
    count as f32r but half the HBM/DMA traffic everywhere.
  - Projection runs 1024-token blocks with 1024-wide moving operands
    (PSUM tiles spanning 2 banks), halving PE instruction count.
  - Softmax denominators no longer burn PE matmul cycles per score chunk:
    a bf16 DVE accumulator (4x mode) sums the exp tiles, one tiny
    ones-matmul per q-block reduces it across partitions.
  - Normalization moved to the sender side of the AllToAll (reciprocal on
    DVE, partition_broadcast on Pool) so the wo stage consumes a2a output
    directly - no post-collective normalize pass on the critical path.
  - Causal mask applied multiplicatively to the bf16 exp tiles (DVE 4x)
    instead of f32 adds on PSUM.
  - V transposes via the DMA XBAR (16-bit transpose) instead of PE.
  - The batch-0 AllToAll overlaps batch-1 attention; sender-side
    normalization leaves only the batch-1 collective exposed.

Distribution (8 NeuronCores, Megatron-style tensor parallel over heads):
  - Each core gets 4 Q heads + its matching 1 KV head (wq/wk/wv output-dim
    sharded). Attention computed per-core in a transposed dataflow
    (head_dim on partitions, tokens on the free dim).
  - Per-batch AllToAll reshards the (already normalized) attention output
    token-parallel; each core then runs wo for its 2x256-token block
    against the full wo, so no AllReduce is needed.
"""

import sys

if "/opt/trn_rl_repo" not in sys.path:
    sys.path.insert(0, "/opt/trn_rl_repo")

import numpy as np
import ml_dtypes

BF16 = ml_dtypes.bfloat16

N_CORES = 8
B, S, D = 2, 2048, 4096
N_HEADS = 32
N_KV_HEADS = 8
HEAD_DIM = 128
H_PER_CORE = N_HEADS // N_CORES          # 4 q heads per core
TOK = B * S                              # 4096 flattened tokens
QKV_M = H_PER_CORE * HEAD_DIM + 2 * HEAD_DIM  # 768 projection rows per core
PROJ_TOK = 512                           # token block in the projection stage
SQ_BLK = 512                             # moving width in attention (2 heads x 256)
A2A_TOK = 256                            # tokens per rank per per-batch AllToAll
NJJ = S // A2A_TOK                       # 8 q-blocks of 256 per batch
N_TCHUNK = S // HEAD_DIM                 # 16 key chunks per batch
SCALE = 1.0 / float(np.sqrt(HEAD_DIM))
NKC = D // 128                           # 32 contraction chunks

# partition permutation for RoPE: pair (even, odd) lives 16 partitions apart
# inside a 32-partition quadrant, so the rotation is a single stream_shuffle.
_P = np.arange(128)
_I_OF_P = 16 * (_P // 32) + (_P % 32) % 16          # rope pair index 0..63
_IS_ODD = (_P % 32) >= 16
PERM = (2 * _I_OF_P + _IS_ODD.astype(np.int64)).astype(np.int64)  # orig row in head block
SHUF_MASK = [(i + 16) % 32 for i in range(32)]

_PROGRAMS = {}


def _build_program(mask_mode):
    """Build + compile the SPMD program. mask_mode in {'causal', 'none', 'general'}."""
    import concourse.bass as bass
    import concourse.mybir as mybir
    import concourse.tile as tile
    from concourse import bacc
    from concourse.masks import make_identity

    f32 = mybir.dt.float32
    bf16 = mybir.dt.bfloat16
    Exp = mybir.ActivationFunctionType.Exp

    nc = bacc.Bacc("TRN2", target_bir_lowering=False, debug=False,
                   num_devices=N_CORES)

    xT = nc.dram_tensor("xT", [D, TOK], bf16, kind="ExternalInput")
    wqkvT = nc.dram_tensor("wqkvT", [D, QKV_M], bf16, kind="ExternalInput")
    # wo pre-tiled on host: [m_chunk, p, k_chunk, m_col] so each stationary
    # column-block DMA reads contiguous lines
    woT4 = nc.dram_tensor("woT4", [NKC, 128, NKC, 128], bf16, kind="ExternalInput")
    cos2 = nc.dram_tensor("cos2", [128, S], bf16, kind="ExternalInput")
    sin2 = nc.dram_tensor("sin2", [128, S], bf16, kind="ExternalInput")
    if mask_mode == "general":
        # additive mask stored transposed: maskT[k_pos, q_pos]
        maskT = nc.dram_tensor("maskT", [S, S], f32, kind="ExternalInput")
    out_d = nc.dram_tensor("out", [D, SQ_BLK], bf16, kind="ExternalOutput")

    xT_t = xT.ap().rearrange("(k p) t -> p k t", p=128)      # [128, 32, TOK]
    wqkvT_t = wqkvT.ap().rearrange("(k p) m -> p k m", p=128)  # [128, 32, 768]

    with tile.TileContext(nc) as tc:
        # at_kt/at_kv/at_q are hoisted alongside the projection pools so the
        # attention preamble DMAs (kT, v transposes, q loads) prefetch during
        # the projection stage instead of serializing at the pool boundary.
        with tc.tile_pool(name="const", bufs=1) as const, \
             tc.tile_pool(name="dram", bufs=1, space="DRAM") as dram, \
             tc.tile_pool(name="at_kt", bufs=2) as at_kt, \
             tc.tile_pool(name="at_kv", bufs=2) as at_kv, \
             tc.tile_pool(name="at_q", bufs=2) as at_q:
            # per-core q/k/v (transposed layout), split per batch
            qd = [dram.tile([H_PER_CORE * 128, S], bf16, name=f"qd{b_}")
                  for b_ in range(B)]
            kd = [dram.tile([128, S], bf16, name=f"kd{b_}") for b_ in range(B)]
            vd = [dram.tile([128, S], bf16, name=f"vd{b_}") for b_ in range(B)]
            # one payload tile per (batch, head-pair): collectives need
            # contiguous inputs, and per-hp collectives let each half ship
            # as soon as its two heads finish
            a2a_in = [[dram.tile([N_CORES, 256, A2A_TOK], bf16,
                                 name=f"a2a_in{b_}_{hp_}") for hp_ in range(2)]
                      for b_ in range(B)]
            a2a_out = [[dram.tile([N_CORES, 256, A2A_TOK], bf16,
                                  name=f"a2a_out{b_}_{hp_}")
                        for hp_ in range(2)] for b_ in range(B)]

            ones_col = const.tile([128, 1], bf16)     # lhsT for column sums
            nc.vector.memset(ones_col[:], 1.0)
            ident = const.tile([128, 128], bf16)      # PE transpose identity
            make_identity(nc, ident[:])
            if mask_mode == "causal":
                # multiplicative 0/1 mask for the diagonal chunk-group:
                # cm[p, a, hh, t] = (t - p >= 128*a), same for both packed
                # heads hh
                cm = const.tile([128, 2, 2, A2A_TOK], bf16, name="cm")
                nc.gpsimd.memset(cm[:], 1.0)
                for a in range(2):
                    for hh in range(2):
                        nc.gpsimd.affine_select(
                            out=cm[:, a, hh, :],
                            in_=cm[:, a, hh, :],
                            pattern=[[1, A2A_TOK]], base=-128 * a,
                            channel_multiplier=-1,
                            compare_op=mybir.AluOpType.is_ge, fill=0.0,
                        )

            def emit_preamble(b):
                """kT / vT / packed-q loads for one batch's attention.
                Called mid-projection for batch 0 so the DMAs overlap the
                remaining projection blocks."""
                kT = at_kt.tile([128, S], bf16, tag="kT", name=f"kT{b}")
                nc.sync.dma_start(kT[:], kd[b][:])
                vT = at_kv.tile([128, S], bf16, tag="vT", name=f"vT{b}")
                nc.sync.dma_start(vT[:], vd[b][:])
                qTps = []
                for hp in range(2):
                    qTp = at_q.tile([128, NJJ, 2, A2A_TOK], bf16, tag="qTp",
                                    name=f"qTp{b}_{hp}")
                    for hh in range(2):
                        nc.sync.dma_start(
                            qTp[:, :, hh, :],
                            qd[b][(2 * hp + hh) * 128:
                                  (2 * hp + hh + 1) * 128, :]
                            .rearrange("p (jj t) -> p jj t", jj=NJJ))
                    qTps.append(qTp)
                return kT, vT, qTps

            pre = {}

            # ---------------- stage 1: fused QKV projection + RoPE ----------------
            # k-outer / m-inner with 6 live PSUM accumulation groups, so the
            # two half-K x tiles (xA, xB) double-buffer against each other.
            n_blk = TOK // PROJ_TOK   # 8
            HK = NKC // 2
            with tc.tile_pool(name="pj_w", bufs=1) as pj_w, \
                 tc.tile_pool(name="pj_x", bufs=3) as pj_x, \
                 tc.tile_pool(name="pj_cs", bufs=2) as pj_cs, \
                 tc.tile_pool(name="pj_t", bufs=2) as pj_t, \
                 tc.tile_pool(name="pj_o", bufs=2) as pj_o, \
                 tc.tile_pool(name="pj_ps", bufs=8, space="PSUM") as pj_ps:
                w_sb = pj_w.tile([128, NKC, QKV_M], bf16)
                x0 = slice(0, PROJ_TOK)
                xA0 = pj_x.tile([128, HK, PROJ_TOK], bf16, tag="xA")
                xB0 = pj_x.tile([128, HK, PROJ_TOK], bf16, tag="xB")
                # first x tile split by k-chunk so matmuls start immediately
                for kw in range(0, HK, 4):
                    nc.sync.dma_start(xA0[:, kw:kw + 4, :],
                                      xT_t[:, kw:kw + 4, x0])
                # split the weight load by k-chunk so the first matmuls can
                # start before the full 6.3MB arrives
                for kw in range(0, NKC, 4):
                    nc.sync.dma_start(w_sb[:, kw:kw + 4, :],
                                      wqkvT_t[:, kw:kw + 4, :])
                    if kw == 0:
                        nc.sync.dma_start(xB0[:], xT_t[:, HK:NKC, x0])
                for n in range(n_blk):
                    s0 = (n * PROJ_TOK) % S  # position within the batch
                    bn = n // (S // PROJ_TOK)  # batch of this token block
                    cols = slice(n * PROJ_TOK, (n + 1) * PROJ_TOK)
                    bcols = slice(s0, s0 + PROJ_TOK)
                    if n == 0:
                        xA, xB = xA0, xB0
                    else:
                        xA = pj_x.tile([128, HK, PROJ_TOK], bf16, tag="xA")
                        xB = pj_x.tile([128, HK, PROJ_TOK], bf16, tag="xB")
                        nc.sync.dma_start(xA[:], xT_t[:, 0:HK, cols])
                        nc.sync.dma_start(xB[:], xT_t[:, HK:NKC, cols])
                    c_sb = pj_cs.tile([128, PROJ_TOK], bf16, tag="c")
                    s_sb = pj_cs.tile([128, PROJ_TOK], bf16, tag="s")
                    nc.sync.dma_start(c_sb[:], cos2.ap()[:, s0:s0 + PROJ_TOK])
                    nc.sync.dma_start(s_sb[:], sin2.ap()[:, s0:s0 + PROJ_TOK])
                    # m-outer / k-inner: each head's full contraction finishes
                    # first, so its RoPE drain overlaps the next head's
                    # matmuls (keeps the block tail short)
                    for m in range(QKV_M // 128):  # q0..q3, k, v
                        ps = pj_ps.tile([128, PROJ_TOK], f32, tag="ps",
                                        name=f"ps_{n}_{m}")
                        for k in range(NKC):
                            xsb = xA if k < HK else xB
                            xi = k if k < HK else k - HK
                            nc.tensor.matmul(
                                ps[:], w_sb[:, k, m * 128:(m + 1) * 128],
                                xsb[:, xi, :],
                                start=(k == 0), stop=(k == NKC - 1))
                        o_sb = pj_o.tile([128, PROJ_TOK], bf16, tag="o")
                        if m < 5:  # rope for q heads + k
                            tmp = pj_t.tile([128, PROJ_TOK], bf16, tag="tmp")
                            rot = pj_t.tile([128, PROJ_TOK], bf16, tag="rot")
                            t1 = pj_t.tile([128, PROJ_TOK], bf16, tag="t1")
                            nc.scalar.copy(tmp[:], ps[:])
                            nc.vector.stream_shuffle(rot[:], tmp[:], SHUF_MASK)
                            nc.vector.tensor_mul(t1[:], tmp[:], c_sb[:])
                            nc.vector.tensor_mul(rot[:], rot[:], s_sb[:])
                            nc.vector.tensor_add(o_sb[:], t1[:], rot[:])
                        else:
                            nc.scalar.copy(o_sb[:], ps[:])
                        if m < 4:
                            dst = qd[bn][m * 128:(m + 1) * 128, bcols]
                        elif m == 4:
                            dst = kd[bn][:, bcols]
                        else:
                            dst = vd[bn][:, bcols]
                        nc.sync.dma_start(dst, o_sb[:])
                    if n == S // PROJ_TOK - 1:
                        # emit batch-0 attention preamble DMAs here so they
                        # queue ahead of the remaining projection stores and
                        # run as soon as batch 0's q/k/v land in DRAM
                        pre[0] = emit_preamble(0)

            # ---------------- stage 2: attention + per-batch AllToAll --------
            # Two q-heads packed side by side in the 512-wide moving operand
            # (2 x 256 tokens); causality handled at 256-token granularity.
            # wo_w is opened before the attention pools so the wo weight
            # prefetch streams during attention / the collectives. The batch-0
            # collective overlaps batch-1 attention; Pool-stream ordering is
            # kept safe by deferring each hp's normalization broadcast to its
            # end (so it is never queued behind a collective it doesn't need).
            with tc.tile_pool(name="wo_w", bufs=4) as wo_w:
                with tc.tile_pool(name="at_e", bufs=6) as at_e, \
                     tc.tile_pool(name="at_acc", bufs=2) as at_acc, \
                     tc.tile_pool(name="at_nrm", bufs=2) as at_nrm, \
                     tc.tile_pool(name="at_o", bufs=3) as at_o, \
                     tc.tile_pool(name="at_mt", bufs=4) as at_mt, \
                     tc.tile_pool(name="ps_s", bufs=2, space="PSUM") as ps_s, \
                     tc.tile_pool(name="ps_av", bufs=2, space="PSUM") as ps_av, \
                     tc.tile_pool(name="ps_sm", bufs=1, space="PSUM") as ps_sm, \
                     tc.tile_pool(name="ps_tr", bufs=1, space="PSUM") as ps_tr:
                    for b in range(B):
                        if b not in pre:
                            pre[b] = emit_preamble(b)
                        kT, vT, qTps = pre[b]
                        v_nat = at_kv.tile([128, N_TCHUNK, 128], bf16,
                                           tag="vn", name=f"vn{b}")
                        for i in range(N_TCHUNK):
                            tp = ps_tr.tile([128, 128], bf16, tag="tr",
                                            name="tp")
                            nc.tensor.transpose(
                                tp[:], vT[:, i * 128:(i + 1) * 128], ident[:])
                            nc.scalar.copy(v_nat[:, i, :], tp[:])
                        for hp in range(2):
                            qTp = qTps[hp]

                            # pending av pair, carried across q-block (jj)
                            # boundaries so the PE stream never drains:
                            # (av_tile, c0, e_tile, is_last, accT, jj)
                            prev = [None]
                            # pending normalization chain, deferred one more
                            # group so the PE isn't gated on the DVE
                            # accumulator finishing at each jj boundary
                            post = [None]

                            def emit_post(b=b, hp=hp):
                                if post[0] is None:
                                    return
                                pav, pacc, pjj = post[0]
                                # the avr copy releases the av PSUM bank
                                # without waiting on the Pool broadcast
                                # (which can queue behind a collective)
                                avr = at_o.tile([128, SQ_BLK], bf16,
                                                tag="avr", name="avr")
                                nc.vector.tensor_copy(avr[:], pav[:])
                                sm = ps_sm.tile([1, SQ_BLK], f32,
                                                tag="sm", name="sm")
                                nc.tensor.matmul(sm[:], ones_col[:],
                                                 pacc[:, 0, :],
                                                 start=True, stop=False)
                                nc.tensor.matmul(sm[:], ones_col[:],
                                                 pacc[:, 1, :],
                                                 start=False, stop=True)
                                rs = at_nrm.tile([1, SQ_BLK], f32,
                                                 tag="rs", name="rs")
                                nc.vector.reciprocal_approx_fast(
                                    out=rs[:], in_=sm[:])
                                rb = at_nrm.tile([128, SQ_BLK], f32,
                                                 tag="rb", name="rb")
                                nc.gpsimd.partition_broadcast(rb[:], rs[:])
                                at = at_o.tile([128, SQ_BLK], bf16,
                                               tag="at", name="at")
                                nc.vector.tensor_mul(at[:], avr[:], rb[:])
                                nc.sync.dma_start(
                                    a2a_in[b][hp][pjj]
                                    .rearrange("(hh p) t -> p hh t", p=128),
                                    at[:].rearrange("p (hh t) -> p hh t",
                                                    hh=2))
                                post[0] = None

                            def emit_av(nxt=None, v_nat=v_nat):
                                if prev[0] is None:
                                    prev[0] = nxt
                                    return
                                pav, pc0, pe, plast, pacc, pjj = prev[0]
                                nc.tensor.matmul(
                                    pav[:], v_nat[:, pc0, :], pe[:, 0, :],
                                    start=(pc0 == 0), stop=False)
                                nc.tensor.matmul(
                                    pav[:], v_nat[:, pc0 + 1, :], pe[:, 1, :],
                                    start=False, stop=plast)
                                emit_post()
                                if plast:
                                    post[0] = (pav, pacc, pjj)
                                prev[0] = nxt

                            # descending jj: the deep-pipeline blocks come
                            # first (better PE ramp), the shallow ones drain
                            # quickly right before the collective fires
                            for jj in reversed(range(NJJ)):
                                G = (jj + 1 if mask_mode == "causal"
                                     else N_TCHUNK // 2)
                                qs = qTp[:, jj, :, :].rearrange(
                                    "p hh t -> p (hh t)")
                                accT = at_acc.tile([128, 2, SQ_BLK], bf16,
                                                   tag="accT")
                                av = ps_av.tile([128, SQ_BLK], f32, tag="av")
                                for g in range(G):
                                    c0 = 2 * g
                                    sp = ps_s.tile([128, 2, SQ_BLK], f32,
                                                   tag="s")
                                    nc.tensor.matmul(
                                        sp[:, 0, :],
                                        kT[:, c0 * 128:(c0 + 1) * 128],
                                        qs, start=True, stop=True)
                                    nc.tensor.matmul(
                                        sp[:, 1, :],
                                        kT[:, (c0 + 1) * 128:(c0 + 2) * 128],
                                        qs, start=True, stop=True)
                                    sp_flat = sp[:].rearrange(
                                        "p a q -> p (a q)")
                                    if mask_mode == "general":
                                        mt = at_mt.tile([128, 2, 2, A2A_TOK],
                                                        f32, tag="mt")
                                        for hh in range(2):
                                            nc.sync.dma_start(
                                                mt[:, :, hh, :],
                                                maskT.ap()[
                                                    c0 * 128:(c0 + 2) * 128,
                                                    jj * A2A_TOK:
                                                    (jj + 1) * A2A_TOK]
                                                .rearrange("(c p) q -> p c q",
                                                           p=128))
                                        nc.vector.tensor_add(
                                            sp_flat, sp_flat,
                                            mt[:].rearrange(
                                                "p c hh q -> p (c hh q)"))
                                    # first group's exp writes the
                                    # accumulator directly
                                    e = (accT if g == 0 else
                                         at_e.tile([128, 2, SQ_BLK], bf16,
                                                   tag="e"))
                                    e_flat = e[:].rearrange("p a q -> p (a q)")
                                    nc.scalar.activation(e_flat, sp_flat, Exp,
                                                         scale=SCALE)
                                    if mask_mode == "causal" and g == G - 1:
                                        nc.vector.tensor_mul(
                                            e_flat, e_flat,
                                            cm[:].rearrange(
                                                "p a hh t -> p (a hh t)"))
                                    # flush the pending av pair BEFORE the
                                    # accumulator update: when g==1 the
                                    # pending e aliases accT and must be
                                    # consumed before accT += e
                                    emit_av((av, c0, e, g == G - 1,
                                             accT, jj))
                                    if g > 0:
                                        nc.vector.tensor_add(accT[:], accT[:],
                                                             e[:])
                            emit_av()
                            emit_post()
                            # per-head-pair collective: ships while the next
                            # hp / batch computes; only the very last one's
                            # half-payload is exposed
                            nc.gpsimd.collective_compute(
                                "AllToAll", mybir.AluOpType.bypass,
                                replica_groups=[list(range(N_CORES))],
                                ins=[a2a_in[b][hp].opt()],
                                outs=[a2a_out[b][hp].opt()],
                            )

                # ------------- stage 3: wo projection (2x256 owned tokens) ----
                with tc.tile_pool(name="wo_a", bufs=1) as wo_a, \
                     tc.tile_pool(name="wo_o", bufs=3) as wo_o, \
                     tc.tile_pool(name="wo_ps", bufs=4, space="PSUM") as wo_ps:
                    # hp0 kv-chunks first: they arrive with the first half of
                    # the last batch's split collective, so wo accumulation
                    # starts while the second half is still in flight
                    korder = [kc for kc in range(NKC) if kc % 4 < 2] + \
                             [kc for kc in range(NKC) if kc % 4 >= 2]
                    a_sb = wo_a.tile([128, NKC, SQ_BLK], bf16)
                    for kc in korder:
                        r_, hh = kc // 4, kc % 4
                        for b in range(B):
                            nc.sync.dma_start(
                                a_sb[:, kc, b * A2A_TOK:(b + 1) * A2A_TOK],
                                a2a_out[b][hh // 2][r_,
                                                    (hh % 2) * 128:
                                                    (hh % 2 + 1) * 128, :])
                    for m in range(NKC):
                        w_sb2 = wo_w.tile([128, NKC, 128], bf16, tag="w")
                        nc.sync.dma_start(w_sb2[:], woT4.ap()[m])
                        ps = wo_ps.tile([128, SQ_BLK], f32, tag="ps")
                        for ki, k in enumerate(korder):
                            nc.tensor.matmul(
                                ps[:], w_sb2[:, k, :], a_sb[:, k, :],
                                start=(ki == 0), stop=(ki == NKC - 1))
                        o_sb = wo_o.tile([128, SQ_BLK], bf16, tag="o")
                        nc.vector.tensor_copy(o_sb[:], ps[:])
                        nc.sync.dma_start(
                            out_d.ap()[m * 128:(m + 1) * 128, :], o_sb[:])

    nc.compile()
    return nc


def _get_program(mask_mode):
    if mask_mode not in _PROGRAMS:
        _PROGRAMS[mask_mode] = _build_program(mask_mode)
    return _PROGRAMS[mask_mode]


def _classify_mask(m2):
    if not m2.any():
        return "none"
    causal_ref = np.triu(np.full((S, S), -1e9, dtype=np.float32), k=1)
    return "causal" if np.array_equal(m2, causal_ref) else "general"


def _prep_inputs(x, freqs_cos, freqs_sin, mask, wq, wk, wv, wo):
    """Host-side sharding / layout prep shared by kernel() and test.py."""
    m2 = np.asarray(mask, np.float32).reshape(S, S)
    mask_mode = _classify_mask(m2)

    xT = np.ascontiguousarray(
        np.asarray(x, np.float32).reshape(TOK, D).T).astype(BF16)
    woT = np.asarray(wo, np.float32).T          # [hd_in, D_out]
    # pre-tile wo for contiguous stationary-block DMAs:
    # woT4[m, p, k, mcol] = woT[k*128+p, m*128+mcol]
    woT4 = np.ascontiguousarray(
        woT.reshape(NKC, 128, NKC, 128).transpose(2, 1, 0, 3)).astype(BF16)

    fc = np.asarray(freqs_cos, np.float32)
    fs = np.asarray(freqs_sin, np.float32)
    cos2 = np.ascontiguousarray(fc.T[_I_OF_P, :]).astype(BF16)    # [128, S]
    sgn = np.where(_IS_ODD, 1.0, -1.0).astype(np.float32)[:, None]
    sin2 = np.ascontiguousarray(fs.T[_I_OF_P, :] * sgn).astype(BF16)

    def permute_heads(w):
        w4 = np.asarray(w, np.float32).reshape(-1, HEAD_DIM, D)
        return w4[:, PERM, :].reshape(-1, D)

    wq_p = permute_heads(wq)
    wk_p = permute_heads(wk)
    wv = np.asarray(wv, np.float32)

    in_maps = []
    for c in range(N_CORES):
        wqkvT = np.ascontiguousarray(np.concatenate(
            [wq_p[c * 512:(c + 1) * 512], wk_p[c * 128:(c + 1) * 128],
             wv[c * 128:(c + 1) * 128]], axis=0).T).astype(BF16)   # [D, 768]
        m = {"xT": xT, "wqkvT": wqkvT, "woT4": woT4, "cos2": cos2, "sin2": sin2}
        if mask_mode == "general":
            m["maskT"] = np.ascontiguousarray(m2.T)
        in_maps.append(m)
    return mask_mode, in_maps


def kernel(x, start_pos, freqs_cos, freqs_sin, mask, cache_k, cache_v,
           wq, wk, wv, wo):
    from concourse.bass_utils import run_bass_kernel_spmd

    assert int(start_pos) == 0, "kernel compiled for start_pos == 0"
    mask_mode, in_maps = _prep_inputs(x, freqs_cos, freqs_sin, mask,
                                      wq, wk, wv, wo)
    nc = _get_program(mask_mode)
    res = run_bass_kernel_spmd(nc, in_maps, list(range(N_CORES)))
    out = np.empty((TOK, D), dtype=np.float32)
    for c in range(N_CORES):
        blk = np.asarray(res.results[c]["out"]).astype(np.float32)  # [D, 512]
        for b in range(B):
            rows = slice(b * S + A2A_TOK * c, b * S + A2A_TOK * (c + 1))
            out[rows, :] = blk[:, b * A2A_TOK:(b + 1) * A2A_TOK].T
    return out.reshape(B, S, D)
